# revision 5
# baseline (speedup 1.0000x reference)
"""CodeGen-style attention block, tensor-parallel over heads on 8 Trainium2 cores.

Strategy (megatron-style):
  - Each core owns 2 of the 16 heads: computes Q/K/V projections for its
    head-slice of w_qkv, runs causal attention for those heads, then applies
    its row-slice of w_out, producing a partial [tokens, H] output.
  - Host sums the 8 partial outputs (the out-proj contraction over heads).

v5 notes:
  - All matmuls are fp16 (fp32 PSUM accumulate): fp32r moving operands stream
    at ~0.55 ns/col on HW while fp16 hits the full 1 col/cycle rate, halves
    every DMA transfer, and enables fast weight loads.
  - Phase 1 is split: 1a projects K+Q over hidden windows 0..7, 1b projects V
    walking windows 7..0 (reusing window 7's SBUF-resident hidden tile).
    The split frees the K/Q weight space during 1b, which buys room to
    preload w_out and the first q tiles while 1b computes - phase 2 then
    starts with zero DMA waits.
  - V is projected directly into [token, dim] layout (hidden chunks
    stationary, w_v moving) - no PE transposes / DVE copies in phase 2.
    Batch 0's V and K land straight in resident SBUF tiles (no DRAM trip).
  - Attention inner loop is software-pipelined two chunks deep: AV for chunk
    kc trails the scores of chunk kc+2, so ScalarE's exp latency never
    reaches the PE - even on the short diagonal sub-tiles.
  - The softmax denominator rides the PE as one fp16 ones-matmul per chunk.
  - The reciprocal uses the ~5x faster Newton-Raphson DVE variant (4e-6 rel
    err, irrelevant vs fp16 rounding).
  - Diagonal k-chunks only compute the live column subrange [i*128, 512).
  - All stores ride the GpSimd SWDGE queue; the Sync HWDGE queue carries
    only loads, so prefetches never sit behind bulk writes.
"""

import sys
import types
from contextlib import ExitStack

import numpy as np

import concourse.bacc as bacc
import concourse.mybir as mybir
import concourse.tile as tile
from concourse.bass_utils import run_bass_kernel_spmd

# bass_utils imports antenv.axon_hooks when tracing is requested via env;
# provide a no-op stub if the module is absent so a stray BASS_TRACE in the
# environment cannot break execution.
try:
    import antenv.axon_hooks  # noqa: F401
except ImportError:
    _stub = types.ModuleType("antenv.axon_hooks")
    _stub.get_axon_ntff_profile_hook = lambda: None
    _stub.set_axon_ntff_profile_hook = lambda h: None
    sys.modules.setdefault("antenv.axon_hooks", _stub)

F32 = mybir.dt.float32
F16 = mybir.dt.float16
AF = mybir.ActivationFunctionType

B, S, H = 2, 2048, 4096
N_HEAD, HEAD_DIM, ROT = 16, 256, 64
MAX_POS = 2048
TOK = B * S            # 4096
N_CORES = 8
HPC = N_HEAD // N_CORES  # heads per core = 2
DPC = HPC * HEAD_DIM     # dims per core = 512
NEG = -30000.0

LAST_EXEC_NS = None
_NC_CACHE = []


def _build():
    nc = bacc.Bacc("TRN2", target_bir_lowering=False, debug=False,
                   num_devices=N_CORES)

    # [w, p, hc*512+t]: hsT window tiles (512 tokens each), per-partition-contiguous
    hst_d = nc.dram_tensor("hst", [8, 128, 32 * 512], F16, kind="ExternalInput")
    # [oc, p, hc*128+d]: per-core w_qkv column-chunks for q (oc 0-3), k (oc 4-7)
    wqkv_d = nc.dram_tensor("wqkv", [8, 128, 32 * 128], F16, kind="ExternalInput")
    # [p, hc*512+v]: per-core w_v slice, hidden-chunk-major (moving operand)
    wv_d = nc.dram_tensor("wv", [128, 32 * 512], F16, kind="ExternalInput")
    # [p, c, n]: per-core w_out row-slice
    wout_d = nc.dram_tensor("wout", [128, 4, H], F16, kind="ExternalInput")
    rope_d = nc.dram_tensor("rope", [128, TOK], F16, kind="ExternalInput")
    rt_d = nc.dram_tensor("rt", [64, 64], F16, kind="ExternalInput")
    onm_d = nc.dram_tensor("onesm", [128, 128], F16, kind="ExternalInput")
    msk_d = nc.dram_tensor("masks", [128, 4, 128], F16, kind="ExternalInput")
    kb_d = nc.dram_tensor("kb", [128, 32], F32, kind="ExternalInput")
    out_d = nc.dram_tensor("out", [TOK, H], F16, kind="ExternalOutput")

    K_OCS = (4, 5, 6, 7)
    Q_OCS = (0, 1, 2, 3)

    with tile.TileContext(nc) as tc:
        with ExitStack() as st0:
            ec0 = st0.enter_context
            dram_pool = ec0(tc.tile_pool(name="dram", bufs=1, space="DRAM"))
            # DRAM intermediates; batch 0's K and V stay on-chip instead.
            qkvT = {}
            for oc in range(8):
                for b in range(2):
                    if oc >= 4 and b == 0:
                        continue
                    qkvT[(oc, b)] = dram_pool.tile(
                        [128, 2048], F16, tag=f"qkvT{oc}_{b}",
                        name=f"qkvT{oc}_{b}")
            vh1_dram = dram_pool.tile([128, 16 * 512], F16, tag="vh1",
                                      name="vh1")
            # persistent SBUF: resident K (b=0 written by 1a), resident V
            # (b=0 written by 1b), small attention constants, and the shared
            # hidden-window pool reused by 1a and 1b
            kpool = ec0(tc.tile_pool(name="kt", bufs=1))
            kts = {}
            for hl in range(2):
                for dc in range(2):
                    kts[(hl, dc)] = kpool.tile(
                        [128, 2048], F16, tag=f"kt{hl}{dc}", name=f"kt{hl}{dc}")
            vhpool = ec0(tc.tile_pool(name="vh", bufs=1))
            vh = vhpool.tile([128, 16 * 512], F16, tag="vha", name="vha")
            c2 = ec0(tc.tile_pool(name="p2c", bufs=1))
            msk_sb = c2.tile([128, 4, 128], F16)
            nc.sync.dma_start(msk_sb[:], msk_d[:])
            kb_sb = c2.tile([128, 32], F32)
            nc.sync.dma_start(kb_sb[:], kb_d[:])
            onm_sb = c2.tile([128, 128], F16)
            nc.sync.dma_start(onm_sb[:], onm_d[:])
            hpool = ec0(tc.tile_pool(name="ht", bufs=2))

            def ht_load(w, strips=1):
                # strip the transfer so the first H-chunks land (and the
                # first matmuls start) before the whole 4MB tile arrives
                t = hpool.tile([128, 32 * 512], F16, name="ht")
                step = 32 // strips
                for s in range(strips):
                    cs = slice(s * step * 512, (s + 1) * step * 512)
                    nc.sync.dma_start(t[:, cs], hst_d[w][:, cs])
                return t

            # ---------------- Phase 1a: K+Q projection + rotary ----------------
            with ExitStack() as st1:
                ec = st1.enter_context
                cpool = ec(tc.tile_pool(name="p1c", bufs=1))
                wpool = ec(tc.tile_pool(name="w", bufs=1))
                spool = ec(tc.tile_pool(name="stage", bufs=4))
                tpool = ec(tc.tile_pool(name="rott", bufs=4))
                apool = ec(tc.tile_pool(name="acc", bufs=4, space="PSUM"))
                rpool = ec(tc.tile_pool(name="rp", bufs=2, space="PSUM"))
                rope_sb = cpool.tile([128, TOK], F16)
                rt_sb = cpool.tile([64, 64], F16)

                wts = {}

                def load_w(oc):
                    wt = wpool.tile([128, 32 * 128], F16, tag=f"w{oc}",
                                    name=f"wt{oc}")
                    nc.sync.dma_start(wt[:], wqkv_d[oc])
                    wts[oc] = wt

                load_w(K_OCS[0])  # first MMs need it
                ht = ht_load(0, 4)
                for oc in K_OCS[1:] + Q_OCS:
                    load_w(oc)
                nc.sync.dma_start(rope_sb[:], rope_d[:])
                nc.sync.dma_start(rt_sb[:], rt_d[:])

                def project(wt, dest, rot, ws):
                    # dest: [128, 512] fp16 slice (stage tile or resident kt)
                    acc = apool.tile([128, 512], F32)
                    for hc in range(32):
                        nc.tensor.matmul(
                            acc[:], wt[:, hc * 128:(hc + 1) * 128],
                            ht[:, hc * 512:(hc + 1) * 512],
                            start=(hc == 0), stop=(hc == 31))
                    nc.scalar.copy(dest[:], acc[:])
                    if rot:
                        # partial rotary on first 64 dims of this head
                        rp = rpool.tile([64, 512], F32)
                        nc.tensor.matmul(rp[:], rt_sb[:], dest[0:64, :])
                        t1 = tpool.tile([64, 512], F16, tag="t1")
                        nc.vector.tensor_mul(
                            t1[:], acc[0:64, :], rope_sb[0:64, ws])
                        t2 = tpool.tile([64, 512], F16, tag="t2")
                        nc.vector.tensor_mul(
                            t2[:], rp[:], rope_sb[64:128, ws])
                        nc.vector.tensor_add(dest[0:64, :], t1[:], t2[:])

                for w in range(8):
                    if w > 0:
                        ht = next_ht
                    b, wo = w // 4, (w % 4) * 512
                    ws = slice(w * 512, (w + 1) * 512)
                    for j, oc in enumerate(K_OCS):
                        if j == 1 and w < 7:
                            # prefetch next window under this one's compute
                            next_ht = ht_load(w + 1, 2 if w < 2 else 1)
                        rot = oc in (4, 6)
                        if b == 0:
                            hl, dc = (oc - 4) // 2, (oc - 4) % 2
                            project(wts[oc], kts[(hl, dc)][:, wo:wo + 512],
                                    rot, ws)
                        else:
                            stage = spool.tile([128, 512], F16)
                            project(wts[oc], stage, rot, ws)
                            nc.gpsimd.dma_start(
                                qkvT[(oc, b)][:, wo:wo + 512], stage[:])
                    for oc in Q_OCS:
                        stage = spool.tile([128, 512], F16)
                        project(wts[oc], stage, oc in (0, 2), ws)
                        nc.gpsimd.dma_start(
                            qkvT[(oc, b)][:, wo:wo + 512], stage[:])
                ht_w7 = ht

            # -------- 1b scope: w_out/q pools live here so their loads run
            # under the V projection --------
            with ExitStack() as stB:
                ecB = stB.enter_context
                c3 = ecB(tc.tile_pool(name="p2w", bufs=1))
                qpool = ecB(tc.tile_pool(name="qq", bufs=2))
                wout_sb = c3.tile([128, 4, H], F16)
                # out-proj weights stream in column chunks during 1b
                for ch in range(8):
                    cs = slice(ch * 512, (ch + 1) * 512)
                    nc.sync.dma_start(wout_sb[:, :, cs], wout_d[:, :, cs])

                # ---------------- Phase 1b: V projection ----------------
                with ExitStack() as st1b:
                    ec = st1b.enter_context
                    wvpool = ec(tc.tile_pool(name="wv", bufs=1))
                    spool2 = ec(tc.tile_pool(name="stage2", bufs=4))
                    apool2 = ec(tc.tile_pool(name="acc2", bufs=4, space="PSUM"))
                    wv_sb = wvpool.tile([128, 32 * 512], F16, name="wv")
                    nc.sync.dma_start(wv_sb[:], wv_d[:])
                    ht = ht_w7  # window 7's hidden tile is still resident
                    for w in range(7, -1, -1):
                        if w != 7:
                            ht = next_ht
                        b = w // 4
                        for tc_ in range(4):
                            if tc_ == 1 and w > 0:
                                next_ht = ht_load(w - 1)
                            acc = apool2.tile([128, 512], F32)
                            for hc in range(32):
                                nc.tensor.matmul(
                                    acc[:],
                                    ht[:, hc * 512 + tc_ * 128:
                                       hc * 512 + (tc_ + 1) * 128],
                                    wv_sb[:, hc * 512:(hc + 1) * 512],
                                    start=(hc == 0), stop=(hc == 31))
                            kc = (w % 4) * 4 + tc_
                            if b == 0:
                                # batch 0's V lands in the resident tile
                                nc.scalar.copy(
                                    vh[:, kc * 512:(kc + 1) * 512], acc[:])
                            else:
                                stage = spool2.tile([128, 512], F16)
                                nc.scalar.copy(stage[:], acc[:])
                                nc.gpsimd.dma_start(
                                    vh1_dram[:, kc * 512:(kc + 1) * 512],
                                    stage[:])

                # ---------------- Phase 2: attention + out-proj ----------------
                with ExitStack() as st2:
                    ec = st2.enter_context
                    expool = ec(tc.tile_pool(name="ex", bufs=6))
                    recpool = ec(tc.tile_pool(name="rec", bufs=2))
                    aopool = ec(tc.tile_pool(name="ao", bufs=2))
                    ospool = ec(tc.tile_pool(name="os", bufs=4))
                    scpool = ec(tc.tile_pool(name="sc", bufs=2, space="PSUM"))
                    avpool = ec(tc.tile_pool(name="av", bufs=1, space="PSUM"))
                    denpool = ec(tc.tile_pool(name="den", bufs=2, space="PSUM"))
                    oppool = ec(tc.tile_pool(name="op", bufs=2, space="PSUM"))

                    def emit_outproj(b, qt, aos):
                        qo = qt * 512
                        for tc_ in range(4):
                            for ht_ in range(8):
                                op = oppool.tile([128, 512], F32, tag="op")
                                for ci, (hl, dc) in enumerate(
                                        ((0, 0), (0, 1), (1, 0), (1, 1))):
                                    nc.tensor.matmul(
                                        op[:],
                                        aos[(hl, dc)][:, tc_ * 128:(tc_ + 1) * 128],
                                        wout_sb[:, 2 * hl + dc,
                                                ht_ * 512:(ht_ + 1) * 512],
                                        start=(ci == 0), stop=(ci == 3))
                                os_ = ospool.tile([128, 512], F16)
                                # split PSUM evacuation across both copy engines
                                if ht_ % 2 == 0:
                                    nc.scalar.copy(os_[:], op[:])
                                else:
                                    nc.vector.tensor_copy(os_[:], op[:])
                                r0 = b * 2048 + qo + tc_ * 128
                                nc.gpsimd.dma_start(
                                    out_d[r0:r0 + 128,
                                          ht_ * 512:(ht_ + 1) * 512],
                                    os_[:])

                    pending = None
                    for b in range(2):
                        if b == 1:
                            nc.sync.dma_start(vh[:], vh1_dram[:])
                            for hl in range(2):
                                for dc in range(2):
                                    nc.sync.dma_start(
                                        kts[(hl, dc)][:],
                                        qkvT[(4 + 2 * hl + dc, b)][:])
                        for qt in range(4):
                            nkc = 4 * qt + 4  # causal: later k-chunks all-masked
                            qo = qt * 512
                            qmap = {}
                            for hl in range(2):
                                for dc in range(2):
                                    q = qpool.tile([128, 512], F16,
                                                   tag=f"q{hl}{dc}")
                                    nc.sync.dma_start(
                                        q[:],
                                        qkvT[(2 * hl + dc, b)][:, qo:qo + 512])
                                    qmap[(hl, dc)] = q
                            aos = {}
                            for hl in range(2):
                                qs = [qmap[(hl, 0)], qmap[(hl, 1)]]
                                av0 = avpool.tile([128, 512], F32, tag="av0")
                                av1 = avpool.tile([128, 512], F32, tag="av1")
                                den = denpool.tile([128, 512], F32)

                                def emit_av(kc, ex, lo):
                                    nc.tensor.matmul(
                                        av0[:, lo:512],
                                        vh[:, kc * 512 + hl * 256:
                                           kc * 512 + hl * 256 + 128],
                                        ex[:, lo:512],
                                        start=(kc == 0), stop=(kc == nkc - 1))
                                    nc.tensor.matmul(
                                        av1[:, lo:512],
                                        vh[:, kc * 512 + hl * 256 + 128:
                                           kc * 512 + hl * 256 + 256],
                                        ex[:, lo:512],
                                        start=(kc == 0), stop=(kc == nkc - 1))
                                    # denominator, pre-broadcast across
                                    # partitions: ones.T @ ex = colsum x128
                                    nc.tensor.matmul(
                                        den[:, lo:512], onm_sb[:], ex[:, lo:512],
                                        start=(kc == 0), stop=(kc == nkc - 1))

                                pend = []
                                for kc in range(nkc):
                                    # diagonal chunks: columns below lo are
                                    # fully masked and skipped
                                    lo = max(0, (kc - 4 * qt) * 128)
                                    sc = scpool.tile([128, 512], F32)
                                    nc.tensor.matmul(
                                        sc[:, lo:512],
                                        kts[(hl, 0)][:, kc * 128:(kc + 1) * 128],
                                        qs[0][:, lo:512], start=True, stop=False)
                                    nc.tensor.matmul(
                                        sc[:, lo:512],
                                        kts[(hl, 1)][:, kc * 128:(kc + 1) * 128],
                                        qs[1][:, lo:512], start=False, stop=True)
                                    if kc >= 4 * qt:
                                        # triangular mask: 128-wide diag strip
                                        nc.vector.tensor_add(
                                            sc[:, lo:lo + 128],
                                            sc[:, lo:lo + 128],
                                            msk_sb[:, kc - 4 * qt, :])
                                    # two-deep software pipeline: AV for chunk
                                    # kc-2 sits behind the scores of chunk kc
                                    if len(pend) >= 2:
                                        emit_av(*pend.pop(0))
                                    ex = expool.tile([128, 512], F16)
                                    nc.scalar.activation(
                                        ex[:, lo:512], sc[:, lo:512], AF.Exp,
                                        scale=1.0 / 16.0,
                                        bias=kb_sb[:, b * 16 + kc:
                                                   b * 16 + kc + 1])
                                    pend.append((kc, ex, lo))
                                for p in pend:
                                    emit_av(*p)
                                # av-bank evacuation split across ScalarE and
                                # DVE so the banks free fast and the next
                                # block's first AV never waits
                                avc0 = aopool.tile([128, 512], F32, bufs=1,
                                                   tag=f"avs{hl}0", name="avc")
                                nc.scalar.copy(avc0[:], av0[:])
                                avc1 = aopool.tile([128, 512], F32, bufs=1,
                                                   tag=f"avs{hl}1", name="avc")
                                nc.vector.tensor_copy(avc1[:], av1[:])
                                rec = recpool.tile([128, 512], F32, tag="rec",
                                                   bufs=1)
                                nc.vector.reciprocal_approx_fast(rec[:], den[:])
                                for dc, avc in ((0, avc0), (1, avc1)):
                                    ao = aopool.tile([128, 512], F16,
                                                     tag=f"ao{hl}{dc}")
                                    nc.vector.tensor_mul(ao[:], avc[:], rec[:])
                                    aos[(hl, dc)] = ao
                            # emit the PREVIOUS block's out-proj here so its
                            # matmuls sit behind this block's attention in PE
                            # program order and never wait on normalization
                            if pending is not None:
                                emit_outproj(*pending)
                            pending = (b, qt, aos)
                    emit_outproj(*pending)
    nc.compile()
    return nc


def _get_nc():
    if not _NC_CACHE:
        _NC_CACHE.append(_build())
    return _NC_CACHE[0]


def _host_prep(hidden_states, position_ids, attention_mask, w_qkv, w_out):
    hid = np.ascontiguousarray(np.asarray(hidden_states, np.float32)).reshape(TOK, H)
    w_qkv = np.asarray(w_qkv, np.float32)
    w_out = np.asarray(w_out, np.float32)
    pos = np.asarray(position_ids).astype(np.int64)
    am = np.asarray(attention_mask).reshape(B, S).astype(bool)

    # hsT window tiles [w, p, hc*512+t]
    hst = np.ascontiguousarray(
        hid.reshape(8, 512, 32, 128).transpose(0, 3, 2, 1)
    ).reshape(8, 128, 32 * 512).astype(np.float16)

    # rotary tables, matching reference.create_sinusoidal_positions
    inv_freq = 1.0 / 10000 ** (np.arange(0, ROT, 2) / ROT)
    si = np.einsum('i,j->ij', np.arange(MAX_POS), inv_freq).astype('float32')
    emb = np.concatenate([np.sin(si), np.cos(si)], axis=-1)  # [2048, 64]
    sincos = emb[pos]                    # [B, S, 64]
    sin_rep = np.repeat(sincos[..., :ROT // 2], 2, axis=2)   # [B, S, 64]
    cos_rep = np.repeat(sincos[..., ROT // 2:], 2, axis=2)
    rope = np.empty((128, TOK), np.float16)
    rope[0:64] = cos_rep.reshape(TOK, 64).T
    rope[64:128] = sin_rep.reshape(TOK, 64).T

    rt = np.zeros((64, 64), np.float16)
    rt[np.arange(1, 64, 2), np.arange(0, 64, 2)] = -1.0
    rt[np.arange(0, 64, 2), np.arange(1, 64, 2)] = 1.0

    onesm = np.ones((128, 128), np.float16)

    # triangular mask for the 128-wide diagonal strip: key partition p vs
    # query column offset within the strip (identical for every strip)
    p_idx = np.arange(128)[:, None, None]
    q_idx = np.arange(128)[None, None, :]
    masks = np.where(p_idx <= q_idx, 0.0, NEG).astype(np.float16)
    masks = np.broadcast_to(masks, (128, 4, 128)).copy()

    kb = np.where(am.reshape(B, 16, 128), 0.0, NEG).astype(
        np.float32).transpose(2, 0, 1).reshape(128, 32)
    kb = np.ascontiguousarray(kb)

    shared = dict(hst=hst, rope=rope, rt=rt, onesm=onesm, masks=masks, kb=kb)

    in_maps = []
    for c in range(N_CORES):
        qk_cols = []
        v_cols = []
        # fused layout per mp-group is (query, value, key)
        for part, dest in ((0, qk_cols), (2, qk_cols), (1, v_cols)):
            for hl in range(HPC):
                h = HPC * c + hl
                base = (h // 4) * 3072 + part * 1024 + (h % 4) * 256
                dest.append(np.arange(base, base + 256))
        qk_cols = np.concatenate(qk_cols)  # [1024] = q(512) | k(512)
        wslice = w_qkv[:, qk_cols]         # [4096, 1024]
        wqkv_prep = np.ascontiguousarray(
            wslice.reshape(32, 128, 8, 128).transpose(2, 1, 0, 3)
        ).reshape(8, 128, 32 * 128).astype(np.float16)
        v_cols = np.concatenate(v_cols)    # [512]
        wv_prep = np.ascontiguousarray(
            w_qkv[:, v_cols].reshape(32, 128, 512).transpose(1, 0, 2)
        ).reshape(128, 32 * 512).astype(np.float16)
        wout_prep = np.ascontiguousarray(
            w_out[c * DPC:(c + 1) * DPC, :].reshape(4, 128, H).transpose(1, 0, 2)
        ).astype(np.float16)
        in_maps.append(dict(shared, wqkv=wqkv_prep, wv=wv_prep, wout=wout_prep))
    return in_maps


def kernel(hidden_states, position_ids, attention_mask, w_qkv, w_out):
    global LAST_EXEC_NS
    nc = _get_nc()
    in_maps = _host_prep(hidden_states, position_ids, attention_mask,
                         w_qkv, w_out)
    res = run_bass_kernel_spmd(nc, in_maps, core_ids=list(range(N_CORES)))
    LAST_EXEC_NS = res.exec_time_ns
    out = res.results[0]["out"].astype(np.float32)
    for c in range(1, N_CORES):
        out = out + res.results[c]["out"].astype(np.float32)
    return out.reshape(B, S, H)


# revision 8
# speedup vs baseline: 1.2187x; 1.2187x over previous
"""CodeGen-style attention block, tensor-parallel over heads on 8 Trainium2 cores.

Strategy (megatron-style):
  - Each core owns 2 of the 16 heads: computes Q/K/V projections for its
    head-slice of w_qkv, runs causal attention for those heads, then applies
    its row-slice of w_out, producing a partial [tokens, H] output.
  - Host sums the 8 partial outputs (the out-proj contraction over heads).

v5 notes:
  - All matmuls are fp16 (fp32 PSUM accumulate): fp32r moving operands stream
    at ~0.55 ns/col on HW while fp16 hits the full 1 col/cycle rate, halves
    every DMA transfer, and enables fast weight loads.
  - Phase 1 is split: 1a projects K+Q over hidden windows 0..7, 1b projects V
    walking windows 7..0 (reusing window 7's SBUF-resident hidden tile).
    The split frees the K/Q weight space during 1b, which buys room to
    preload w_out and the first q tiles while 1b computes - phase 2 then
    starts with zero DMA waits.
  - V is projected directly into [token, dim] layout (hidden chunks
    stationary, w_v moving) - no PE transposes / DVE copies in phase 2.
    Batch 0's V and K land straight in resident SBUF tiles (no DRAM trip).
  - Attention inner loop is software-pipelined two chunks deep: AV for chunk
    kc trails the scores of chunk kc+2, so ScalarE's exp latency never
    reaches the PE - even on the short diagonal sub-tiles.
  - The softmax denominator rides the PE as one fp16 ones-matmul per chunk.
  - The reciprocal uses the ~5x faster Newton-Raphson DVE variant (4e-6 rel
    err, irrelevant vs fp16 rounding).
  - Diagonal k-chunks only compute the live column subrange [i*128, 512).
  - All stores ride the GpSimd SWDGE queue; the Sync HWDGE queue carries
    only loads, so prefetches never sit behind bulk writes.
"""

import sys
import types
from contextlib import ExitStack

import numpy as np

import concourse.bacc as bacc
import concourse.mybir as mybir
import concourse.tile as tile
from concourse.bass_utils import run_bass_kernel_spmd

# bass_utils imports antenv.axon_hooks when tracing is requested via env;
# provide a no-op stub if the module is absent so a stray BASS_TRACE in the
# environment cannot break execution.
try:
    import antenv.axon_hooks  # noqa: F401
except ImportError:
    _stub = types.ModuleType("antenv.axon_hooks")
    _stub.get_axon_ntff_profile_hook = lambda: None
    _stub.set_axon_ntff_profile_hook = lambda h: None
    sys.modules.setdefault("antenv.axon_hooks", _stub)

F32 = mybir.dt.float32
F16 = mybir.dt.float16
AF = mybir.ActivationFunctionType

B, S, H = 2, 2048, 4096
N_HEAD, HEAD_DIM, ROT = 16, 256, 64
MAX_POS = 2048
TOK = B * S            # 4096
N_CORES = 8
HPC = N_HEAD // N_CORES  # heads per core = 2
DPC = HPC * HEAD_DIM     # dims per core = 512
NEG = -30000.0

LAST_EXEC_NS = None
_NC_CACHE = []


def _build():
    nc = bacc.Bacc("TRN2", target_bir_lowering=False, debug=False,
                   num_devices=N_CORES)

    # [w, p, hc*512+t]: hsT window tiles (512 tokens each), per-partition-contiguous
    hst_d = nc.dram_tensor("hst", [8, 128, 32 * 512], F16, kind="ExternalInput")
    # [oc, p, hc*128+d]: per-core w_qkv column-chunks for q (oc 0-3), k (oc 4-7)
    wqkv_d = nc.dram_tensor("wqkv", [8, 128, 32 * 128], F16, kind="ExternalInput")
    # [p, hc*512+v]: per-core w_v slice, hidden-chunk-major (moving operand)
    wv_d = nc.dram_tensor("wv", [128, 32 * 512], F16, kind="ExternalInput")
    # [p, c, n]: per-core w_out row-slice
    wout_d = nc.dram_tensor("wout", [128, 4, H], F16, kind="ExternalInput")
    rope_d = nc.dram_tensor("rope", [128, TOK], F16, kind="ExternalInput")
    rt_d = nc.dram_tensor("rt", [64, 64], F16, kind="ExternalInput")
    onm_d = nc.dram_tensor("onesm", [128, 128], F16, kind="ExternalInput")
    msk_d = nc.dram_tensor("masks", [128, 4, 128], F16, kind="ExternalInput")
    kb_d = nc.dram_tensor("kb", [128, 32], F32, kind="ExternalInput")
    out_d = nc.dram_tensor("out", [TOK, H], F16, kind="ExternalOutput")

    K_OCS = (4, 5, 6, 7)
    Q_OCS = (0, 1, 2, 3)

    with tile.TileContext(nc) as tc:
        with ExitStack() as st0:
            ec0 = st0.enter_context
            dram_pool = ec0(tc.tile_pool(name="dram", bufs=1, space="DRAM"))
            # DRAM intermediates; batch 0's K and V stay on-chip instead.
            qkvT = {}
            for oc in range(8):
                for b in range(2):
                    if oc >= 4 and b == 0:
                        continue
                    qkvT[(oc, b)] = dram_pool.tile(
                        [128, 2048], F16, tag=f"qkvT{oc}_{b}",
                        name=f"qkvT{oc}_{b}")
            vh1_dram = dram_pool.tile([128, 16 * 512], F16, tag="vh1",
                                      name="vh1")
            # persistent SBUF: resident K (b=0 written by 1a), resident V
            # (b=0 written by 1b), small attention constants, and the shared
            # hidden-window pool reused by 1a and 1b
            kpool = ec0(tc.tile_pool(name="kt", bufs=1))
            kts = {}
            for hl in range(2):
                for dc in range(2):
                    kts[(hl, dc)] = kpool.tile(
                        [128, 2048], F16, tag=f"kt{hl}{dc}", name=f"kt{hl}{dc}")
            vhpool = ec0(tc.tile_pool(name="vh", bufs=1))
            vh = vhpool.tile([128, 16 * 512], F16, tag="vha", name="vha")
            c2 = ec0(tc.tile_pool(name="p2c", bufs=1))
            msk_sb = c2.tile([128, 4, 128], F16)
            nc.sync.dma_start(msk_sb[:], msk_d[:])
            kb_sb = c2.tile([128, 32], F32)
            nc.sync.dma_start(kb_sb[:], kb_d[:])
            onm_sb = c2.tile([128, 128], F16)
            nc.sync.dma_start(onm_sb[:], onm_d[:])
            # w_v lives in the persistent scope so its load streams in under
            # phase 1a's compute (a 1b-scoped tile would WAR-wait on 1a's
            # freed pools and stall 1b's first matmuls)
            wvpool = ec0(tc.tile_pool(name="wv", bufs=1))
            wv_sb = wvpool.tile([128, 32 * 512], F16, name="wv")
            hpool = ec0(tc.tile_pool(name="ht", bufs=2))

            def ht_load(w, strips=1):
                # strip the transfer so the first H-chunks land (and the
                # first matmuls start) before the whole 4MB tile arrives
                t = hpool.tile([128, 32 * 512], F16, name="ht")
                step = 32 // strips
                for s in range(strips):
                    cs = slice(s * step * 512, (s + 1) * step * 512)
                    nc.sync.dma_start(t[:, cs], hst_d[w][:, cs])
                return t

            # ---------------- Phase 1a: K+Q projection + rotary ----------------
            with ExitStack() as st1:
                ec = st1.enter_context
                cpool = ec(tc.tile_pool(name="p1c", bufs=1))
                wpool = ec(tc.tile_pool(name="w", bufs=1))
                spool = ec(tc.tile_pool(name="stage", bufs=3))
                tpool = ec(tc.tile_pool(name="rott", bufs=1))
                apool = ec(tc.tile_pool(name="acc", bufs=4, space="PSUM"))
                rpool = ec(tc.tile_pool(name="rp", bufs=2, space="PSUM"))
                rope_sb = cpool.tile([128, TOK], F16)
                rt_sb = cpool.tile([64, 64], F16)

                wts = {}

                def load_w(oc):
                    wt = wpool.tile([128, 32 * 128], F16, tag=f"w{oc}",
                                    name=f"wt{oc}")
                    nc.sync.dma_start(wt[:], wqkv_d[oc])
                    wts[oc] = wt

                load_w(K_OCS[0])  # first MMs need it
                ht = ht_load(0, 4)
                for oc in K_OCS[1:] + Q_OCS:
                    load_w(oc)
                nc.sync.dma_start(wv_sb[:], wv_d[:])
                nc.sync.dma_start(rope_sb[:], rope_d[:])
                nc.sync.dma_start(rt_sb[:], rt_d[:])

                def project(wt, dest, rot, ws):
                    # dest: [128, 512] fp16 slice (stage tile or resident kt)
                    acc = apool.tile([128, 512], F32)
                    for hc in range(32):
                        nc.tensor.matmul(
                            acc[:], wt[:, hc * 128:(hc + 1) * 128],
                            ht[:, hc * 512:(hc + 1) * 512],
                            start=(hc == 0), stop=(hc == 31))
                    nc.scalar.copy(dest[:], acc[:])
                    if rot:
                        # partial rotary on first 64 dims of this head
                        rp = rpool.tile([64, 512], F32)
                        nc.tensor.matmul(rp[:], rt_sb[:], dest[0:64, :])
                        t1 = tpool.tile([64, 512], F16, tag="t1")
                        nc.vector.tensor_mul(
                            t1[:], acc[0:64, :], rope_sb[0:64, ws])
                        t2 = tpool.tile([64, 512], F16, tag="t2")
                        nc.vector.tensor_mul(
                            t2[:], rp[:], rope_sb[64:128, ws])
                        nc.vector.tensor_add(dest[0:64, :], t1[:], t2[:])

                for w in range(8):
                    if w > 0:
                        ht = next_ht
                    b, wo = w // 4, (w % 4) * 512
                    ws = slice(w * 512, (w + 1) * 512)
                    for j, oc in enumerate(K_OCS):
                        if j == 1 and w < 7:
                            # prefetch next window under this one's compute
                            next_ht = ht_load(w + 1, 2 if w < 2 else 1)
                        rot = oc in (4, 6)
                        if b == 0:
                            hl, dc = (oc - 4) // 2, (oc - 4) % 2
                            project(wts[oc], kts[(hl, dc)][:, wo:wo + 512],
                                    rot, ws)
                        else:
                            stage = spool.tile([128, 512], F16)
                            project(wts[oc], stage, rot, ws)
                            nc.gpsimd.dma_start(
                                qkvT[(oc, b)][:, wo:wo + 512], stage[:])
                    for oc in Q_OCS:
                        stage = spool.tile([128, 512], F16)
                        project(wts[oc], stage, oc in (0, 2), ws)
                        nc.gpsimd.dma_start(
                            qkvT[(oc, b)][:, wo:wo + 512], stage[:])
                ht_w7 = ht

            # -------- 1b scope: w_out/q pools live here so their loads run
            # under the V projection --------
            with ExitStack() as stB:
                ecB = stB.enter_context
                c3 = ecB(tc.tile_pool(name="p2w", bufs=1))
                qpool = ecB(tc.tile_pool(name="qq", bufs=2))
                wout_sb = c3.tile([128, 4, H], F16)
                # out-proj weights stream in column chunks during 1b
                for ch in range(8):
                    cs = slice(ch * 512, (ch + 1) * 512)
                    nc.sync.dma_start(wout_sb[:, :, cs], wout_d[:, :, cs])

                # ---------------- Phase 1b: V projection ----------------
                with ExitStack() as st1b:
                    ec = st1b.enter_context
                    spool2 = ec(tc.tile_pool(name="stage2", bufs=4))
                    apool2 = ec(tc.tile_pool(name="acc2", bufs=4, space="PSUM"))
                    ht = ht_w7  # window 7's hidden tile is still resident
                    for w in range(7, -1, -1):
                        if w != 7:
                            ht = next_ht
                        b = w // 4
                        for tc_ in range(4):
                            if tc_ == 1 and w > 0:
                                next_ht = ht_load(w - 1)
                            acc = apool2.tile([128, 512], F32)
                            for hc in range(32):
                                nc.tensor.matmul(
                                    acc[:],
                                    ht[:, hc * 512 + tc_ * 128:
                                       hc * 512 + (tc_ + 1) * 128],
                                    wv_sb[:, hc * 512:(hc + 1) * 512],
                                    start=(hc == 0), stop=(hc == 31))
                            kc = (w % 4) * 4 + tc_
                            if b == 0:
                                # batch 0's V lands in the resident tile
                                nc.scalar.copy(
                                    vh[:, kc * 512:(kc + 1) * 512], acc[:])
                            else:
                                stage = spool2.tile([128, 512], F16)
                                nc.scalar.copy(stage[:], acc[:])
                                nc.gpsimd.dma_start(
                                    vh1_dram[:, kc * 512:(kc + 1) * 512],
                                    stage[:])

                # ---------------- Phase 2: attention + out-proj ----------------
                with ExitStack() as st2:
                    ec = st2.enter_context
                    expool = ec(tc.tile_pool(name="ex", bufs=6))
                    recpool = ec(tc.tile_pool(name="rec", bufs=2))
                    aopool = ec(tc.tile_pool(name="ao", bufs=2))
                    ospool = ec(tc.tile_pool(name="os", bufs=4))
                    scpool = ec(tc.tile_pool(name="sc", bufs=2, space="PSUM"))
                    avpool = ec(tc.tile_pool(name="av", bufs=1, space="PSUM"))
                    denpool = ec(tc.tile_pool(name="den", bufs=2, space="PSUM"))
                    oppool = ec(tc.tile_pool(name="op", bufs=2, space="PSUM"))

                    def emit_outproj(b, qt, aos):
                        qo = qt * 512
                        for tc_ in range(4):
                            for ht_ in range(8):
                                op = oppool.tile([128, 512], F32, tag="op")
                                for ci, (hl, dc) in enumerate(
                                        ((0, 0), (0, 1), (1, 0), (1, 1))):
                                    nc.tensor.matmul(
                                        op[:],
                                        aos[(hl, dc)][:, tc_ * 128:(tc_ + 1) * 128],
                                        wout_sb[:, 2 * hl + dc,
                                                ht_ * 512:(ht_ + 1) * 512],
                                        start=(ci == 0), stop=(ci == 3))
                                os_ = ospool.tile([128, 512], F16)
                                # split PSUM evacuation across both copy engines
                                if ht_ % 2 == 0:
                                    nc.scalar.copy(os_[:], op[:])
                                else:
                                    nc.vector.tensor_copy(os_[:], op[:])
                                r0 = b * 2048 + qo + tc_ * 128
                                nc.gpsimd.dma_start(
                                    out_d[r0:r0 + 128,
                                          ht_ * 512:(ht_ + 1) * 512],
                                    os_[:])

                    pending = None
                    for b in range(2):
                        if b == 1:
                            nc.sync.dma_start(vh[:], vh1_dram[:])
                            for hl in range(2):
                                for dc in range(2):
                                    nc.sync.dma_start(
                                        kts[(hl, dc)][:],
                                        qkvT[(4 + 2 * hl + dc, b)][:])
                        for qt in range(4):
                            nkc = 4 * qt + 4  # causal: later k-chunks all-masked
                            qo = qt * 512
                            qmap = {}
                            for hl in range(2):
                                for dc in range(2):
                                    q = qpool.tile([128, 512], F16,
                                                   tag=f"q{hl}{dc}")
                                    nc.sync.dma_start(
                                        q[:],
                                        qkvT[(2 * hl + dc, b)][:, qo:qo + 512])
                                    qmap[(hl, dc)] = q
                            aos = {}
                            for hl in range(2):
                                qs = [qmap[(hl, 0)], qmap[(hl, 1)]]
                                av0 = avpool.tile([128, 512], F32, tag="av0")
                                av1 = avpool.tile([128, 512], F32, tag="av1")
                                den = denpool.tile([128, 512], F32)

                                def emit_av(kc, ex, lo):
                                    nc.tensor.matmul(
                                        av0[:, lo:512],
                                        vh[:, kc * 512 + hl * 256:
                                           kc * 512 + hl * 256 + 128],
                                        ex[:, lo:512],
                                        start=(kc == 0), stop=(kc == nkc - 1))
                                    nc.tensor.matmul(
                                        av1[:, lo:512],
                                        vh[:, kc * 512 + hl * 256 + 128:
                                           kc * 512 + hl * 256 + 256],
                                        ex[:, lo:512],
                                        start=(kc == 0), stop=(kc == nkc - 1))
                                    # denominator, pre-broadcast across
                                    # partitions: ones.T @ ex = colsum x128
                                    nc.tensor.matmul(
                                        den[:, lo:512], onm_sb[:], ex[:, lo:512],
                                        start=(kc == 0), stop=(kc == nkc - 1))

                                pend = []
                                for kc in range(nkc):
                                    # diagonal chunks: columns below lo are
                                    # fully masked and skipped
                                    lo = max(0, (kc - 4 * qt) * 128)
                                    sc = scpool.tile([128, 512], F32)
                                    nc.tensor.matmul(
                                        sc[:, lo:512],
                                        kts[(hl, 0)][:, kc * 128:(kc + 1) * 128],
                                        qs[0][:, lo:512], start=True, stop=False)
                                    nc.tensor.matmul(
                                        sc[:, lo:512],
                                        kts[(hl, 1)][:, kc * 128:(kc + 1) * 128],
                                        qs[1][:, lo:512], start=False, stop=True)
                                    if kc >= 4 * qt:
                                        # triangular mask: 128-wide diag strip
                                        nc.vector.tensor_add(
                                            sc[:, lo:lo + 128],
                                            sc[:, lo:lo + 128],
                                            msk_sb[:, kc - 4 * qt, :])
                                    # two-deep software pipeline: AV for chunk
                                    # kc-2 sits behind the scores of chunk kc
                                    if len(pend) >= 2:
                                        emit_av(*pend.pop(0))
                                    ex = expool.tile([128, 512], F16)
                                    nc.scalar.activation(
                                        ex[:, lo:512], sc[:, lo:512], AF.Exp,
                                        scale=1.0 / 16.0,
                                        bias=kb_sb[:, b * 16 + kc:
                                                   b * 16 + kc + 1])
                                    pend.append((kc, ex, lo))
                                for p in pend:
                                    emit_av(*p)
                                # av-bank evacuation split across ScalarE and
                                # DVE so the banks free fast and the next
                                # block's first AV never waits
                                avc0 = aopool.tile([128, 512], F32, bufs=1,
                                                   tag=f"avs{hl}0", name="avc")
                                nc.scalar.copy(avc0[:], av0[:])
                                avc1 = aopool.tile([128, 512], F32, bufs=1,
                                                   tag=f"avs{hl}1", name="avc")
                                nc.vector.tensor_copy(avc1[:], av1[:])
                                rec = recpool.tile([128, 512], F32, tag="rec",
                                                   bufs=1)
                                nc.vector.reciprocal_approx_fast(rec[:], den[:])
                                for dc, avc in ((0, avc0), (1, avc1)):
                                    ao = aopool.tile([128, 512], F16,
                                                     tag=f"ao{hl}{dc}")
                                    nc.vector.tensor_mul(ao[:], avc[:], rec[:])
                                    aos[(hl, dc)] = ao
                            # emit the PREVIOUS block's out-proj here so its
                            # matmuls sit behind this block's attention in PE
                            # program order and never wait on normalization
                            if pending is not None:
                                emit_outproj(*pending)
                            pending = (b, qt, aos)
                    emit_outproj(*pending)
    nc.compile()
    return nc


def _get_nc():
    if not _NC_CACHE:
        _NC_CACHE.append(_build())
    return _NC_CACHE[0]


def _host_prep(hidden_states, position_ids, attention_mask, w_qkv, w_out):
    hid = np.ascontiguousarray(np.asarray(hidden_states, np.float32)).reshape(TOK, H)
    w_qkv = np.asarray(w_qkv, np.float32)
    w_out = np.asarray(w_out, np.float32)
    pos = np.asarray(position_ids).astype(np.int64)
    am = np.asarray(attention_mask).reshape(B, S).astype(bool)

    # hsT window tiles [w, p, hc*512+t]
    hst = np.ascontiguousarray(
        hid.reshape(8, 512, 32, 128).transpose(0, 3, 2, 1)
    ).reshape(8, 128, 32 * 512).astype(np.float16)

    # rotary tables, matching reference.create_sinusoidal_positions
    inv_freq = 1.0 / 10000 ** (np.arange(0, ROT, 2) / ROT)
    si = np.einsum('i,j->ij', np.arange(MAX_POS), inv_freq).astype('float32')
    emb = np.concatenate([np.sin(si), np.cos(si)], axis=-1)  # [2048, 64]
    sincos = emb[pos]                    # [B, S, 64]
    sin_rep = np.repeat(sincos[..., :ROT // 2], 2, axis=2)   # [B, S, 64]
    cos_rep = np.repeat(sincos[..., ROT // 2:], 2, axis=2)
    rope = np.empty((128, TOK), np.float16)
    rope[0:64] = cos_rep.reshape(TOK, 64).T
    rope[64:128] = sin_rep.reshape(TOK, 64).T

    rt = np.zeros((64, 64), np.float16)
    rt[np.arange(1, 64, 2), np.arange(0, 64, 2)] = -1.0
    rt[np.arange(0, 64, 2), np.arange(1, 64, 2)] = 1.0

    onesm = np.ones((128, 128), np.float16)

    # triangular mask for the 128-wide diagonal strip: key partition p vs
    # query column offset within the strip (identical for every strip)
    p_idx = np.arange(128)[:, None, None]
    q_idx = np.arange(128)[None, None, :]
    masks = np.where(p_idx <= q_idx, 0.0, NEG).astype(np.float16)
    masks = np.broadcast_to(masks, (128, 4, 128)).copy()

    kb = np.where(am.reshape(B, 16, 128), 0.0, NEG).astype(
        np.float32).transpose(2, 0, 1).reshape(128, 32)
    kb = np.ascontiguousarray(kb)

    shared = dict(hst=hst, rope=rope, rt=rt, onesm=onesm, masks=masks, kb=kb)

    in_maps = []
    for c in range(N_CORES):
        qk_cols = []
        v_cols = []
        # fused layout per mp-group is (query, value, key)
        for part, dest in ((0, qk_cols), (2, qk_cols), (1, v_cols)):
            for hl in range(HPC):
                h = HPC * c + hl
                base = (h // 4) * 3072 + part * 1024 + (h % 4) * 256
                dest.append(np.arange(base, base + 256))
        qk_cols = np.concatenate(qk_cols)  # [1024] = q(512) | k(512)
        wslice = w_qkv[:, qk_cols]         # [4096, 1024]
        wqkv_prep = np.ascontiguousarray(
            wslice.reshape(32, 128, 8, 128).transpose(2, 1, 0, 3)
        ).reshape(8, 128, 32 * 128).astype(np.float16)
        v_cols = np.concatenate(v_cols)    # [512]
        wv_prep = np.ascontiguousarray(
            w_qkv[:, v_cols].reshape(32, 128, 512).transpose(1, 0, 2)
        ).reshape(128, 32 * 512).astype(np.float16)
        wout_prep = np.ascontiguousarray(
            w_out[c * DPC:(c + 1) * DPC, :].reshape(4, 128, H).transpose(1, 0, 2)
        ).astype(np.float16)
        in_maps.append(dict(shared, wqkv=wqkv_prep, wv=wv_prep, wout=wout_prep))
    return in_maps


def kernel(hidden_states, position_ids, attention_mask, w_qkv, w_out):
    global LAST_EXEC_NS
    nc = _get_nc()
    in_maps = _host_prep(hidden_states, position_ids, attention_mask,
                         w_qkv, w_out)
    res = run_bass_kernel_spmd(nc, in_maps, core_ids=list(range(N_CORES)))
    LAST_EXEC_NS = res.exec_time_ns
    out = res.results[0]["out"].astype(np.float32)
    for c in range(1, N_CORES):
        out = out + res.results[c]["out"].astype(np.float32)
    return out.reshape(B, S, H)


# revision 9
# speedup vs baseline: 1.2252x; 1.0054x over previous
"""CodeGen-style attention block, tensor-parallel over heads on 8 Trainium2 cores.

Strategy (megatron-style):
  - Each core owns 2 of the 16 heads: computes Q/K/V projections for its
    head-slice of w_qkv, runs causal attention for those heads, then applies
    its row-slice of w_out, producing a partial [tokens, H] output.
  - Host sums the 8 partial outputs (the out-proj contraction over heads).

v5 notes:
  - All matmuls are fp16 (fp32 PSUM accumulate): fp32r moving operands stream
    at ~0.55 ns/col on HW while fp16 hits the full 1 col/cycle rate, halves
    every DMA transfer, and enables fast weight loads.
  - Phase 1 is split: 1a projects K+Q over hidden windows 0..7, 1b projects V
    walking windows 7..0 (reusing window 7's SBUF-resident hidden tile).
    The split frees the K/Q weight space during 1b, which buys room to
    preload w_out and the first q tiles while 1b computes - phase 2 then
    starts with zero DMA waits.
  - V is projected directly into [token, dim] layout (hidden chunks
    stationary, w_v moving) - no PE transposes / DVE copies in phase 2.
    Batch 0's V and K land straight in resident SBUF tiles (no DRAM trip).
  - Attention inner loop is software-pipelined two chunks deep: AV for chunk
    kc trails the scores of chunk kc+2, so ScalarE's exp latency never
    reaches the PE - even on the short diagonal sub-tiles.
  - The softmax denominator rides the PE as one fp16 ones-matmul per chunk.
  - The reciprocal uses the ~5x faster Newton-Raphson DVE variant (4e-6 rel
    err, irrelevant vs fp16 rounding).
  - Diagonal k-chunks only compute the live column subrange [i*128, 512).
  - All stores ride the GpSimd SWDGE queue; the Sync HWDGE queue carries
    only loads, so prefetches never sit behind bulk writes.
"""

import sys
import types
from contextlib import ExitStack

import numpy as np

import concourse.bacc as bacc
import concourse.mybir as mybir
import concourse.tile as tile
from concourse.bass_utils import run_bass_kernel_spmd

# bass_utils imports antenv.axon_hooks when tracing is requested via env;
# provide a no-op stub if the module is absent so a stray BASS_TRACE in the
# environment cannot break execution.
try:
    import antenv.axon_hooks  # noqa: F401
except ImportError:
    _stub = types.ModuleType("antenv.axon_hooks")
    _stub.get_axon_ntff_profile_hook = lambda: None
    _stub.set_axon_ntff_profile_hook = lambda h: None
    sys.modules.setdefault("antenv.axon_hooks", _stub)

F32 = mybir.dt.float32
F16 = mybir.dt.float16
AF = mybir.ActivationFunctionType

B, S, H = 2, 2048, 4096
N_HEAD, HEAD_DIM, ROT = 16, 256, 64
MAX_POS = 2048
TOK = B * S            # 4096
N_CORES = 8
HPC = N_HEAD // N_CORES  # heads per core = 2
DPC = HPC * HEAD_DIM     # dims per core = 512
NEG = -30000.0

LAST_EXEC_NS = None
_NC_CACHE = []


def _build():
    nc = bacc.Bacc("TRN2", target_bir_lowering=False, debug=False,
                   num_devices=N_CORES)

    # [w, p, hc*512+t]: hsT window tiles (512 tokens each), per-partition-contiguous
    hst_d = nc.dram_tensor("hst", [8, 128, 32 * 512], F16, kind="ExternalInput")
    # [oc, p, hc*128+d]: per-core w_qkv column-chunks for q (oc 0-3), k (oc 4-7)
    wqkv_d = nc.dram_tensor("wqkv", [8, 128, 32 * 128], F16, kind="ExternalInput")
    # [p, hc*512+v]: per-core w_v slice, hidden-chunk-major (moving operand)
    wv_d = nc.dram_tensor("wv", [128, 32 * 512], F16, kind="ExternalInput")
    # [p, c, n]: per-core w_out row-slice
    wout_d = nc.dram_tensor("wout", [128, 4, H], F16, kind="ExternalInput")
    rope_d = nc.dram_tensor("rope", [128, TOK], F16, kind="ExternalInput")
    rt_d = nc.dram_tensor("rt", [64, 64], F16, kind="ExternalInput")
    onm_d = nc.dram_tensor("onesm", [128, 128], F16, kind="ExternalInput")
    msk_d = nc.dram_tensor("masks", [128, 4, 128], F16, kind="ExternalInput")
    kb_d = nc.dram_tensor("kb", [128, 32], F32, kind="ExternalInput")
    out_d = nc.dram_tensor("out", [TOK, H], F16, kind="ExternalOutput")

    K_OCS = (4, 5, 6, 7)
    Q_OCS = (0, 1, 2, 3)

    with tile.TileContext(nc) as tc:
        with ExitStack() as st0:
            ec0 = st0.enter_context
            dram_pool = ec0(tc.tile_pool(name="dram", bufs=1, space="DRAM"))
            # DRAM intermediates; batch 0's K and V stay on-chip instead.
            qkvT = {}
            for oc in range(8):
                for b in range(2):
                    if oc >= 4 and b == 0:
                        continue
                    qkvT[(oc, b)] = dram_pool.tile(
                        [128, 2048], F16, tag=f"qkvT{oc}_{b}",
                        name=f"qkvT{oc}_{b}")
            vh1_dram = dram_pool.tile([128, 16 * 512], F16, tag="vh1",
                                      name="vh1")
            # persistent SBUF: resident K (b=0 written by 1a), resident V
            # (b=0 written by 1b), small attention constants, and the shared
            # hidden-window pool reused by 1a and 1b
            kpool = ec0(tc.tile_pool(name="kt", bufs=1))
            kts = {}
            for hl in range(2):
                for dc in range(2):
                    kts[(hl, dc)] = kpool.tile(
                        [128, 2048], F16, tag=f"kt{hl}{dc}", name=f"kt{hl}{dc}")
            vhpool = ec0(tc.tile_pool(name="vh", bufs=1))
            vh = vhpool.tile([128, 16 * 512], F16, tag="vha", name="vha")
            c2 = ec0(tc.tile_pool(name="p2c", bufs=1))
            msk_sb = c2.tile([128, 4, 128], F16)
            nc.sync.dma_start(msk_sb[:], msk_d[:])
            kb_sb = c2.tile([128, 32], F32)
            nc.sync.dma_start(kb_sb[:], kb_d[:])
            onm_sb = c2.tile([128, 128], F16)
            nc.sync.dma_start(onm_sb[:], onm_d[:])
            # w_v lives in the persistent scope so its load streams in under
            # phase 1a's compute (a 1b-scoped tile would WAR-wait on 1a's
            # freed pools and stall 1b's first matmuls)
            wvpool = ec0(tc.tile_pool(name="wv", bufs=1))
            wv_sb = wvpool.tile([128, 32 * 512], F16, name="wv")
            hpool = ec0(tc.tile_pool(name="ht", bufs=2))

            def ht_load(w, strips=1):
                # strip the transfer so the first H-chunks land (and the
                # first matmuls start) before the whole 4MB tile arrives
                t = hpool.tile([128, 32 * 512], F16, name="ht")
                step = 32 // strips
                for s in range(strips):
                    cs = slice(s * step * 512, (s + 1) * step * 512)
                    nc.sync.dma_start(t[:, cs], hst_d[w][:, cs])
                return t

            # ---------------- Phase 1a: K+Q projection + rotary ----------------
            with ExitStack() as st1:
                ec = st1.enter_context
                cpool = ec(tc.tile_pool(name="p1c", bufs=1))
                wpool = ec(tc.tile_pool(name="w", bufs=1))
                spool = ec(tc.tile_pool(name="stage", bufs=3))
                tpool = ec(tc.tile_pool(name="rott", bufs=1))
                apool = ec(tc.tile_pool(name="acc", bufs=4, space="PSUM"))
                rpool = ec(tc.tile_pool(name="rp", bufs=2, space="PSUM"))
                rope_sb = cpool.tile([128, TOK], F16)
                rt_sb = cpool.tile([64, 64], F16)

                wts = {}

                def load_w(oc):
                    wt = wpool.tile([128, 32 * 128], F16, tag=f"w{oc}",
                                    name=f"wt{oc}")
                    nc.sync.dma_start(wt[:], wqkv_d[oc])
                    wts[oc] = wt

                load_w(K_OCS[0])  # first MMs need it
                ht = ht_load(0, 4)
                for oc in K_OCS[1:] + Q_OCS:
                    load_w(oc)
                nc.sync.dma_start(wv_sb[:], wv_d[:])
                nc.sync.dma_start(rope_sb[:], rope_d[:])
                nc.sync.dma_start(rt_sb[:], rt_d[:])

                def project(wt, dest, rot, ws):
                    # dest: [128, 512] fp16 slice (stage tile or resident kt)
                    acc = apool.tile([128, 512], F32)
                    for hc in range(32):
                        nc.tensor.matmul(
                            acc[:], wt[:, hc * 128:(hc + 1) * 128],
                            ht[:, hc * 512:(hc + 1) * 512],
                            start=(hc == 0), stop=(hc == 31))
                    nc.scalar.copy(dest[:], acc[:])
                    if rot:
                        # partial rotary on first 64 dims of this head
                        rp = rpool.tile([64, 512], F32)
                        nc.tensor.matmul(rp[:], rt_sb[:], dest[0:64, :])
                        t1 = tpool.tile([64, 512], F16, tag="t1")
                        nc.vector.tensor_mul(
                            t1[:], acc[0:64, :], rope_sb[0:64, ws])
                        t2 = tpool.tile([64, 512], F16, tag="t2")
                        nc.vector.tensor_mul(
                            t2[:], rp[:], rope_sb[64:128, ws])
                        nc.vector.tensor_add(dest[0:64, :], t1[:], t2[:])

                for w in range(8):
                    if w > 0:
                        ht = next_ht
                    b, wo = w // 4, (w % 4) * 512
                    ws = slice(w * 512, (w + 1) * 512)
                    for j, oc in enumerate(K_OCS):
                        if j == 1 and w < 7:
                            # prefetch next window under this one's compute
                            next_ht = ht_load(w + 1, 2 if w < 2 else 1)
                        rot = oc in (4, 6)
                        if b == 0:
                            hl, dc = (oc - 4) // 2, (oc - 4) % 2
                            project(wts[oc], kts[(hl, dc)][:, wo:wo + 512],
                                    rot, ws)
                        else:
                            stage = spool.tile([128, 512], F16)
                            project(wts[oc], stage, rot, ws)
                            nc.gpsimd.dma_start(
                                qkvT[(oc, b)][:, wo:wo + 512], stage[:])
                    for oc in Q_OCS:
                        stage = spool.tile([128, 512], F16)
                        project(wts[oc], stage, oc in (0, 2), ws)
                        nc.gpsimd.dma_start(
                            qkvT[(oc, b)][:, wo:wo + 512], stage[:])
                ht_w7 = ht

            # -------- 1b scope: w_out/q pools live here so their loads run
            # under the V projection --------
            with ExitStack() as stB:
                ecB = stB.enter_context
                c3 = ecB(tc.tile_pool(name="p2w", bufs=1))
                qpool = ecB(tc.tile_pool(name="qq", bufs=2))
                wout_sb = c3.tile([128, 4, H], F16)
                # out-proj weights stream in column chunks during 1b
                for ch in range(8):
                    cs = slice(ch * 512, (ch + 1) * 512)
                    nc.sync.dma_start(wout_sb[:, :, cs], wout_d[:, :, cs])

                # ---------------- Phase 1b: V projection ----------------
                with ExitStack() as st1b:
                    ec = st1b.enter_context
                    spool2 = ec(tc.tile_pool(name="stage2", bufs=4))
                    apool2 = ec(tc.tile_pool(name="acc2", bufs=4, space="PSUM"))
                    ht = ht_w7  # window 7's hidden tile is still resident
                    for w in range(7, -1, -1):
                        if w != 7:
                            ht = next_ht
                        b = w // 4
                        for tc_ in range(4):
                            if tc_ == 1 and w > 0:
                                next_ht = ht_load(w - 1)
                            acc = apool2.tile([128, 512], F32)
                            for hc in range(32):
                                nc.tensor.matmul(
                                    acc[:],
                                    ht[:, hc * 512 + tc_ * 128:
                                       hc * 512 + (tc_ + 1) * 128],
                                    wv_sb[:, hc * 512:(hc + 1) * 512],
                                    start=(hc == 0), stop=(hc == 31))
                            kc = (w % 4) * 4 + tc_
                            if b == 0:
                                # batch 0's V lands in the resident tile
                                nc.scalar.copy(
                                    vh[:, kc * 512:(kc + 1) * 512], acc[:])
                            else:
                                stage = spool2.tile([128, 512], F16)
                                nc.scalar.copy(stage[:], acc[:])
                                nc.gpsimd.dma_start(
                                    vh1_dram[:, kc * 512:(kc + 1) * 512],
                                    stage[:])

                # ---------------- Phase 2: attention + out-proj ----------------
                with ExitStack() as st2:
                    ec = st2.enter_context
                    expool = ec(tc.tile_pool(name="ex", bufs=6))
                    recpool = ec(tc.tile_pool(name="rec", bufs=2))
                    aopool = ec(tc.tile_pool(name="ao", bufs=2))
                    ospool = ec(tc.tile_pool(name="os", bufs=4))
                    scpool = ec(tc.tile_pool(name="sc", bufs=2, space="PSUM"))
                    avpool = ec(tc.tile_pool(name="av", bufs=1, space="PSUM"))
                    denpool = ec(tc.tile_pool(name="den", bufs=1, space="PSUM"))
                    oppool = ec(tc.tile_pool(name="op", bufs=3, space="PSUM"))

                    def emit_outproj(b, qt, aos):
                        qo = qt * 512
                        for tc_ in range(4):
                            for ht_ in range(8):
                                op = oppool.tile([128, 512], F32, tag="op")
                                for ci, (hl, dc) in enumerate(
                                        ((0, 0), (0, 1), (1, 0), (1, 1))):
                                    nc.tensor.matmul(
                                        op[:],
                                        aos[(hl, dc)][:, tc_ * 128:(tc_ + 1) * 128],
                                        wout_sb[:, 2 * hl + dc,
                                                ht_ * 512:(ht_ + 1) * 512],
                                        start=(ci == 0), stop=(ci == 3))
                                os_ = ospool.tile([128, 512], F16)
                                # split PSUM evacuation 5:3 across the copy
                                # engines (ScalarE takes more; DVE runs the
                                # reciprocal + normalize chain)
                                if ht_ % 8 in (1, 4, 7):
                                    nc.vector.tensor_copy(os_[:], op[:])
                                else:
                                    nc.scalar.copy(os_[:], op[:])
                                r0 = b * 2048 + qo + tc_ * 128
                                # output writes ride the scalar HWDGE queue:
                                # faster than SWDGE (kills the end-of-kernel
                                # drain) and never blocks the load queue
                                nc.scalar.dma_start(
                                    out_d[r0:r0 + 128,
                                          ht_ * 512:(ht_ + 1) * 512],
                                    os_[:])

                    pending = None
                    for b in range(2):
                        if b == 1:
                            nc.sync.dma_start(vh[:], vh1_dram[:])
                            for hl in range(2):
                                for dc in range(2):
                                    nc.sync.dma_start(
                                        kts[(hl, dc)][:],
                                        qkvT[(4 + 2 * hl + dc, b)][:])
                        for qt in range(4):
                            nkc = 4 * qt + 4  # causal: later k-chunks all-masked
                            qo = qt * 512
                            qmap = {}
                            for hl in range(2):
                                for dc in range(2):
                                    q = qpool.tile([128, 512], F16,
                                                   tag=f"q{hl}{dc}")
                                    nc.sync.dma_start(
                                        q[:],
                                        qkvT[(2 * hl + dc, b)][:, qo:qo + 512])
                                    qmap[(hl, dc)] = q
                            aos = {}
                            for hl in range(2):
                                qs = [qmap[(hl, 0)], qmap[(hl, 1)]]
                                av0 = avpool.tile([128, 512], F32, tag="av0")
                                av1 = avpool.tile([128, 512], F32, tag="av1")
                                den = denpool.tile([128, 512], F32)

                                def emit_av(kc, ex, lo):
                                    nc.tensor.matmul(
                                        av0[:, lo:512],
                                        vh[:, kc * 512 + hl * 256:
                                           kc * 512 + hl * 256 + 128],
                                        ex[:, lo:512],
                                        start=(kc == 0), stop=(kc == nkc - 1))
                                    nc.tensor.matmul(
                                        av1[:, lo:512],
                                        vh[:, kc * 512 + hl * 256 + 128:
                                           kc * 512 + hl * 256 + 256],
                                        ex[:, lo:512],
                                        start=(kc == 0), stop=(kc == nkc - 1))
                                    # denominator, pre-broadcast across
                                    # partitions: ones.T @ ex = colsum x128
                                    nc.tensor.matmul(
                                        den[:, lo:512], onm_sb[:], ex[:, lo:512],
                                        start=(kc == 0), stop=(kc == nkc - 1))

                                pend = []
                                for kc in range(nkc):
                                    # diagonal chunks: columns below lo are
                                    # fully masked and skipped
                                    lo = max(0, (kc - 4 * qt) * 128)
                                    sc = scpool.tile([128, 512], F32)
                                    nc.tensor.matmul(
                                        sc[:, lo:512],
                                        kts[(hl, 0)][:, kc * 128:(kc + 1) * 128],
                                        qs[0][:, lo:512], start=True, stop=False)
                                    nc.tensor.matmul(
                                        sc[:, lo:512],
                                        kts[(hl, 1)][:, kc * 128:(kc + 1) * 128],
                                        qs[1][:, lo:512], start=False, stop=True)
                                    if kc >= 4 * qt:
                                        # triangular mask: 128-wide diag strip
                                        nc.vector.tensor_add(
                                            sc[:, lo:lo + 128],
                                            sc[:, lo:lo + 128],
                                            msk_sb[:, kc - 4 * qt, :])
                                    # two-deep software pipeline: AV for chunk
                                    # kc-2 sits behind the scores of chunk kc
                                    if len(pend) >= 2:
                                        emit_av(*pend.pop(0))
                                    ex = expool.tile([128, 512], F16)
                                    nc.scalar.activation(
                                        ex[:, lo:512], sc[:, lo:512], AF.Exp,
                                        scale=1.0 / 16.0,
                                        bias=kb_sb[:, b * 16 + kc:
                                                   b * 16 + kc + 1])
                                    pend.append((kc, ex, lo))
                                for p in pend:
                                    emit_av(*p)
                                # av-bank evacuation split across ScalarE and
                                # DVE so the banks free fast and the next
                                # block's first AV never waits
                                avc0 = aopool.tile([128, 512], F32, bufs=1,
                                                   tag=f"avs{hl}0", name="avc")
                                nc.scalar.copy(avc0[:], av0[:])
                                avc1 = aopool.tile([128, 512], F32, bufs=1,
                                                   tag=f"avs{hl}1", name="avc")
                                nc.vector.tensor_copy(avc1[:], av1[:])
                                rec = recpool.tile([128, 512], F32, tag="rec",
                                                   bufs=1)
                                nc.vector.reciprocal_approx_fast(rec[:], den[:])
                                for dc, avc in ((0, avc0), (1, avc1)):
                                    ao = aopool.tile([128, 512], F16,
                                                     tag=f"ao{hl}{dc}")
                                    nc.vector.tensor_mul(ao[:], avc[:], rec[:])
                                    aos[(hl, dc)] = ao
                            # emit the PREVIOUS block's out-proj here so its
                            # matmuls sit behind this block's attention in PE
                            # program order and never wait on normalization
                            if pending is not None:
                                emit_outproj(*pending)
                            pending = (b, qt, aos)
                    emit_outproj(*pending)
    nc.compile()
    return nc


def _get_nc():
    if not _NC_CACHE:
        _NC_CACHE.append(_build())
    return _NC_CACHE[0]


def _host_prep(hidden_states, position_ids, attention_mask, w_qkv, w_out):
    hid = np.ascontiguousarray(np.asarray(hidden_states, np.float32)).reshape(TOK, H)
    w_qkv = np.asarray(w_qkv, np.float32)
    w_out = np.asarray(w_out, np.float32)
    pos = np.asarray(position_ids).astype(np.int64)
    am = np.asarray(attention_mask).reshape(B, S).astype(bool)

    # hsT window tiles [w, p, hc*512+t]
    hst = np.ascontiguousarray(
        hid.reshape(8, 512, 32, 128).transpose(0, 3, 2, 1)
    ).reshape(8, 128, 32 * 512).astype(np.float16)

    # rotary tables, matching reference.create_sinusoidal_positions
    inv_freq = 1.0 / 10000 ** (np.arange(0, ROT, 2) / ROT)
    si = np.einsum('i,j->ij', np.arange(MAX_POS), inv_freq).astype('float32')
    emb = np.concatenate([np.sin(si), np.cos(si)], axis=-1)  # [2048, 64]
    sincos = emb[pos]                    # [B, S, 64]
    sin_rep = np.repeat(sincos[..., :ROT // 2], 2, axis=2)   # [B, S, 64]
    cos_rep = np.repeat(sincos[..., ROT // 2:], 2, axis=2)
    rope = np.empty((128, TOK), np.float16)
    rope[0:64] = cos_rep.reshape(TOK, 64).T
    rope[64:128] = sin_rep.reshape(TOK, 64).T

    rt = np.zeros((64, 64), np.float16)
    rt[np.arange(1, 64, 2), np.arange(0, 64, 2)] = -1.0
    rt[np.arange(0, 64, 2), np.arange(1, 64, 2)] = 1.0

    onesm = np.ones((128, 128), np.float16)

    # triangular mask for the 128-wide diagonal strip: key partition p vs
    # query column offset within the strip (identical for every strip)
    p_idx = np.arange(128)[:, None, None]
    q_idx = np.arange(128)[None, None, :]
    masks = np.where(p_idx <= q_idx, 0.0, NEG).astype(np.float16)
    masks = np.broadcast_to(masks, (128, 4, 128)).copy()

    kb = np.where(am.reshape(B, 16, 128), 0.0, NEG).astype(
        np.float32).transpose(2, 0, 1).reshape(128, 32)
    kb = np.ascontiguousarray(kb)

    shared = dict(hst=hst, rope=rope, rt=rt, onesm=onesm, masks=masks, kb=kb)

    in_maps = []
    for c in range(N_CORES):
        qk_cols = []
        v_cols = []
        # fused layout per mp-group is (query, value, key)
        for part, dest in ((0, qk_cols), (2, qk_cols), (1, v_cols)):
            for hl in range(HPC):
                h = HPC * c + hl
                base = (h // 4) * 3072 + part * 1024 + (h % 4) * 256
                dest.append(np.arange(base, base + 256))
        qk_cols = np.concatenate(qk_cols)  # [1024] = q(512) | k(512)
        wslice = w_qkv[:, qk_cols]         # [4096, 1024]
        wqkv_prep = np.ascontiguousarray(
            wslice.reshape(32, 128, 8, 128).transpose(2, 1, 0, 3)
        ).reshape(8, 128, 32 * 128).astype(np.float16)
        v_cols = np.concatenate(v_cols)    # [512]
        wv_prep = np.ascontiguousarray(
            w_qkv[:, v_cols].reshape(32, 128, 512).transpose(1, 0, 2)
        ).reshape(128, 32 * 512).astype(np.float16)
        wout_prep = np.ascontiguousarray(
            w_out[c * DPC:(c + 1) * DPC, :].reshape(4, 128, H).transpose(1, 0, 2)
        ).astype(np.float16)
        in_maps.append(dict(shared, wqkv=wqkv_prep, wv=wv_prep, wout=wout_prep))
    return in_maps


def kernel(hidden_states, position_ids, attention_mask, w_qkv, w_out):
    global LAST_EXEC_NS
    nc = _get_nc()
    in_maps = _host_prep(hidden_states, position_ids, attention_mask,
                         w_qkv, w_out)
    res = run_bass_kernel_spmd(nc, in_maps, core_ids=list(range(N_CORES)))
    LAST_EXEC_NS = res.exec_time_ns
    out = res.results[0]["out"].astype(np.float32)
    for c in range(1, N_CORES):
        out = out + res.results[c]["out"].astype(np.float32)
    return out.reshape(B, S, H)


# revision 12
# speedup vs baseline: 1.2430x; 1.0145x over previous
"""CodeGen-style attention block, tensor-parallel over heads on 8 Trainium2 cores.

Strategy (megatron-style):
  - Each core owns 2 of the 16 heads: computes Q/K/V projections for its
    head-slice of w_qkv, runs causal attention for those heads, then applies
    its row-slice of w_out, producing a partial [tokens, H] output.
  - Host sums the 8 partial outputs (the out-proj contraction over heads).

v5 notes:
  - All matmuls are fp16 (fp32 PSUM accumulate): fp32r moving operands stream
    at ~0.55 ns/col on HW while fp16 hits the full 1 col/cycle rate, halves
    every DMA transfer, and enables fast weight loads.
  - Phase 1 is split: 1a projects K+Q over hidden windows 0..7, 1b projects V
    walking windows 7..0 (reusing window 7's SBUF-resident hidden tile).
    The split frees the K/Q weight space during 1b, which buys room to
    preload w_out and the first q tiles while 1b computes - phase 2 then
    starts with zero DMA waits.
  - V is projected directly into [token, dim] layout (hidden chunks
    stationary, w_v moving) - no PE transposes / DVE copies in phase 2.
    Batch 0's V and K land straight in resident SBUF tiles (no DRAM trip).
  - Attention inner loop is software-pipelined two chunks deep: AV for chunk
    kc trails the scores of chunk kc+2, so ScalarE's exp latency never
    reaches the PE - even on the short diagonal sub-tiles.
  - The softmax denominator rides the PE as one fp16 ones-matmul per chunk.
  - The reciprocal uses the ~5x faster Newton-Raphson DVE variant (4e-6 rel
    err, irrelevant vs fp16 rounding).
  - Diagonal k-chunks only compute the live column subrange [i*128, 512).
  - All stores ride the GpSimd SWDGE queue; the Sync HWDGE queue carries
    only loads, so prefetches never sit behind bulk writes.
"""

import sys
import types
from contextlib import ExitStack

import numpy as np

import concourse.bacc as bacc
import concourse.mybir as mybir
import concourse.tile as tile
from concourse.bass_utils import run_bass_kernel_spmd

# bass_utils imports antenv.axon_hooks when tracing is requested via env;
# provide a no-op stub if the module is absent so a stray BASS_TRACE in the
# environment cannot break execution.
try:
    import antenv.axon_hooks  # noqa: F401
except ImportError:
    _stub = types.ModuleType("antenv.axon_hooks")
    _stub.get_axon_ntff_profile_hook = lambda: None
    _stub.set_axon_ntff_profile_hook = lambda h: None
    sys.modules.setdefault("antenv.axon_hooks", _stub)

F32 = mybir.dt.float32
F16 = mybir.dt.float16
AF = mybir.ActivationFunctionType

B, S, H = 2, 2048, 4096
N_HEAD, HEAD_DIM, ROT = 16, 256, 64
MAX_POS = 2048
TOK = B * S            # 4096
N_CORES = 8
HPC = N_HEAD // N_CORES  # heads per core = 2
DPC = HPC * HEAD_DIM     # dims per core = 512
NEG = -30000.0

LAST_EXEC_NS = None
_NC_CACHE = []


def _build():
    nc = bacc.Bacc("TRN2", target_bir_lowering=False, debug=False,
                   num_devices=N_CORES)

    # [w, p, hc*512+t]: hsT window tiles (512 tokens each), per-partition-contiguous
    hst_d = nc.dram_tensor("hst", [8, 128, 32 * 512], F16, kind="ExternalInput")
    # [oc, p, hc*128+d]: per-core w_qkv column-chunks for q (oc 0-3), k (oc 4-7)
    wqkv_d = nc.dram_tensor("wqkv", [8, 128, 32 * 128], F16, kind="ExternalInput")
    # [p, hc*512+v]: per-core w_v slice, hidden-chunk-major (moving operand)
    wv_d = nc.dram_tensor("wv", [128, 32 * 512], F16, kind="ExternalInput")
    # [p, c, n]: per-core w_out row-slice
    wout_d = nc.dram_tensor("wout", [128, 4, H], F16, kind="ExternalInput")
    rope_d = nc.dram_tensor("rope", [128, TOK], F16, kind="ExternalInput")
    rt_d = nc.dram_tensor("rt", [64, 64], F16, kind="ExternalInput")
    onm_d = nc.dram_tensor("onesm", [128, 128], F16, kind="ExternalInput")
    msk_d = nc.dram_tensor("masks", [128, 4, 128], F16, kind="ExternalInput")
    kb_d = nc.dram_tensor("kb", [128, 32], F32, kind="ExternalInput")
    out_d = nc.dram_tensor("out", [TOK, H], F16, kind="ExternalOutput")

    K_OCS = (4, 5, 6, 7)
    Q_OCS = (0, 1, 2, 3)

    with tile.TileContext(nc) as tc:
        with ExitStack() as st0:
            ec0 = st0.enter_context
            dram_pool = ec0(tc.tile_pool(name="dram", bufs=1, space="DRAM"))
            # DRAM intermediates; batch 0's K and V stay on-chip instead.
            qkvT = {}
            for oc in range(8):
                for b in range(2):
                    if oc >= 4 and b == 0:
                        continue
                    qkvT[(oc, b)] = dram_pool.tile(
                        [128, 2048], F16, tag=f"qkvT{oc}_{b}",
                        name=f"qkvT{oc}_{b}")
            vh1_dram = dram_pool.tile([128, 16 * 512], F16, tag="vh1",
                                      name="vh1")
            # persistent SBUF: resident K (b=0 written by 1a), resident V
            # (b=0 written by 1b), small attention constants, and the shared
            # hidden-window pool reused by 1a and 1b
            kpool = ec0(tc.tile_pool(name="kt", bufs=1))
            kts = {}
            for hl in range(2):
                for dc in range(2):
                    kts[(hl, dc)] = kpool.tile(
                        [128, 2048], F16, tag=f"kt{hl}{dc}", name=f"kt{hl}{dc}")
            vhpool = ec0(tc.tile_pool(name="vh", bufs=1))
            vh = vhpool.tile([128, 16 * 512], F16, tag="vha", name="vha")
            c2 = ec0(tc.tile_pool(name="p2c", bufs=1))
            msk_sb = c2.tile([128, 4, 128], F16)
            nc.sync.dma_start(msk_sb[:], msk_d[:])
            kb_sb = c2.tile([128, 32], F32)
            nc.sync.dma_start(kb_sb[:], kb_d[:])
            onm_sb = c2.tile([128, 128], F16)
            nc.sync.dma_start(onm_sb[:], onm_d[:])
            # w_v lives in the persistent scope so its load streams in under
            # phase 1a's compute (a 1b-scoped tile would WAR-wait on 1a's
            # freed pools and stall 1b's first matmuls)
            wvpool = ec0(tc.tile_pool(name="wv", bufs=1))
            wv_sb = wvpool.tile([128, 32 * 512], F16, name="wv")
            hpool = ec0(tc.tile_pool(name="ht", bufs=2))

            def ht_load(w, strips=1):
                # strip the transfer so the first H-chunks land (and the
                # first matmuls start) before the whole 4MB tile arrives
                t = hpool.tile([128, 32 * 512], F16, name="ht")
                step = 32 // strips
                for s in range(strips):
                    cs = slice(s * step * 512, (s + 1) * step * 512)
                    nc.sync.dma_start(t[:, cs], hst_d[w][:, cs])
                return t

            # ---------------- Phase 1a: K+Q projection + rotary ----------------
            with ExitStack() as st1:
                ec = st1.enter_context
                cpool = ec(tc.tile_pool(name="p1c", bufs=1))
                wpool = ec(tc.tile_pool(name="w", bufs=1))
                spool = ec(tc.tile_pool(name="stage", bufs=3))
                tpool = ec(tc.tile_pool(name="rott", bufs=1))
                apool = ec(tc.tile_pool(name="acc", bufs=4, space="PSUM"))
                rpool = ec(tc.tile_pool(name="rp", bufs=2, space="PSUM"))
                rope_sb = cpool.tile([128, TOK], F16)
                rt_sb = cpool.tile([64, 64], F16)

                wts = {}

                def load_w(oc, strips=1):
                    wt = wpool.tile([128, 32 * 128], F16, tag=f"w{oc}",
                                    name=f"wt{oc}")
                    step = 32 // strips
                    for s in range(strips):
                        cs = slice(s * step * 128, (s + 1) * step * 128)
                        nc.sync.dma_start(wt[:, cs], wqkv_d[oc][:, cs])
                    wts[oc] = wt

                load_w(K_OCS[0], 4)  # first MMs need it
                ht = ht_load(0, 8)
                for oc in K_OCS[1:] + Q_OCS:
                    load_w(oc)
                nc.sync.dma_start(wv_sb[:], wv_d[:])
                nc.sync.dma_start(rope_sb[:], rope_d[:])
                nc.sync.dma_start(rt_sb[:], rt_d[:])

                def project(wt, dest, rot, ws):
                    # dest: [128, 512] fp16 slice (stage tile or resident kt)
                    acc = apool.tile([128, 512], F32)
                    for hc in range(32):
                        nc.tensor.matmul(
                            acc[:], wt[:, hc * 128:(hc + 1) * 128],
                            ht[:, hc * 512:(hc + 1) * 512],
                            start=(hc == 0), stop=(hc == 31))
                    nc.scalar.copy(dest[:], acc[:])
                    if rot:
                        # partial rotary on first 64 dims of this head
                        rp = rpool.tile([64, 512], F32)
                        nc.tensor.matmul(rp[:], rt_sb[:], dest[0:64, :])
                        t1 = tpool.tile([64, 512], F16, tag="t1")
                        nc.vector.tensor_mul(
                            t1[:], acc[0:64, :], rope_sb[0:64, ws])
                        t2 = tpool.tile([64, 512], F16, tag="t2")
                        nc.vector.tensor_mul(
                            t2[:], rp[:], rope_sb[64:128, ws])
                        nc.vector.tensor_add(dest[0:64, :], t1[:], t2[:])

                for w in range(8):
                    if w > 0:
                        ht = next_ht
                    b, wo = w // 4, (w % 4) * 512
                    ws = slice(w * 512, (w + 1) * 512)
                    for j, oc in enumerate(K_OCS):
                        if j == 1 and w < 7:
                            # prefetch next window under this one's compute
                            next_ht = ht_load(w + 1, 2 if w < 2 else 1)
                        rot = oc in (4, 6)
                        if b == 0:
                            hl, dc = (oc - 4) // 2, (oc - 4) % 2
                            project(wts[oc], kts[(hl, dc)][:, wo:wo + 512],
                                    rot, ws)
                        else:
                            stage = spool.tile([128, 512], F16)
                            project(wts[oc], stage, rot, ws)
                            nc.gpsimd.dma_start(
                                qkvT[(oc, b)][:, wo:wo + 512], stage[:])
                    for oc in Q_OCS:
                        stage = spool.tile([128, 512], F16)
                        project(wts[oc], stage, oc in (0, 2), ws)
                        nc.gpsimd.dma_start(
                            qkvT[(oc, b)][:, wo:wo + 512], stage[:])
                ht_w7 = ht

            # -------- 1b scope: w_out/q pools live here so their loads run
            # under the V projection --------
            with ExitStack() as stB:
                ecB = stB.enter_context
                c3 = ecB(tc.tile_pool(name="p2w", bufs=1))
                qpool = ecB(tc.tile_pool(name="qq", bufs=2))
                wout_sb = c3.tile([128, 4, H], F16)
                # out-proj weights stream in column chunks during 1b
                for ch in range(8):
                    cs = slice(ch * 512, (ch + 1) * 512)
                    nc.sync.dma_start(wout_sb[:, :, cs], wout_d[:, :, cs])

                # ---------------- Phase 1b: V projection ----------------
                with ExitStack() as st1b:
                    ec = st1b.enter_context
                    spool2 = ec(tc.tile_pool(name="stage2", bufs=4))
                    apool2 = ec(tc.tile_pool(name="acc2", bufs=4, space="PSUM"))
                    ht = ht_w7  # window 7's hidden tile is still resident
                    for w in range(7, -1, -1):
                        if w != 7:
                            ht = next_ht
                        b = w // 4
                        for tc_ in range(4):
                            if tc_ == 1 and w > 0:
                                next_ht = ht_load(w - 1)
                            acc = apool2.tile([128, 512], F32)
                            for hc in range(32):
                                nc.tensor.matmul(
                                    acc[:],
                                    ht[:, hc * 512 + tc_ * 128:
                                       hc * 512 + (tc_ + 1) * 128],
                                    wv_sb[:, hc * 512:(hc + 1) * 512],
                                    start=(hc == 0), stop=(hc == 31))
                            kc = (w % 4) * 4 + tc_
                            if b == 0:
                                # batch 0's V lands in the resident tile
                                nc.scalar.copy(
                                    vh[:, kc * 512:(kc + 1) * 512], acc[:])
                            else:
                                stage = spool2.tile([128, 512], F16)
                                nc.scalar.copy(stage[:], acc[:])
                                nc.gpsimd.dma_start(
                                    vh1_dram[:, kc * 512:(kc + 1) * 512],
                                    stage[:])

                # ---------------- Phase 2: attention + out-proj ----------------
                with ExitStack() as st2:
                    ec = st2.enter_context
                    expool = ec(tc.tile_pool(name="ex", bufs=4))
                    recpool = ec(tc.tile_pool(name="rec", bufs=1))
                    aopool = ec(tc.tile_pool(name="ao", bufs=2))
                    ospool = ec(tc.tile_pool(name="os", bufs=2))
                    scpool = ec(tc.tile_pool(name="sc", bufs=2, space="PSUM"))
                    avpool = ec(tc.tile_pool(name="av", bufs=1, space="PSUM"))
                    denpool = ec(tc.tile_pool(name="den", bufs=1, space="PSUM"))
                    oppool = ec(tc.tile_pool(name="op", bufs=3, space="PSUM"))

                    def emit_outproj(b, qt, aos):
                        qo = qt * 512
                        for tc_ in range(4):
                            # full 4096-wide row block staged in SBUF so the
                            # write is ONE descriptor (a per-chunk dma_start
                            # costs ~615ns of engine-sequencer time each)
                            os_ = ospool.tile([128, 4096], F16, tag="os")
                            for ht_ in range(8):
                                op = oppool.tile([128, 512], F32, tag="op")
                                for ci, (hl, dc) in enumerate(
                                        ((0, 0), (0, 1), (1, 0), (1, 1))):
                                    nc.tensor.matmul(
                                        op[:],
                                        aos[(hl, dc)][:, tc_ * 128:(tc_ + 1) * 128],
                                        wout_sb[:, 2 * hl + dc,
                                                ht_ * 512:(ht_ + 1) * 512],
                                        start=(ci == 0), stop=(ci == 3))
                                # split PSUM evacuation 5:3 across the copy
                                # engines (ScalarE takes more; DVE runs the
                                # reciprocal + normalize chain)
                                osl = os_[:, ht_ * 512:(ht_ + 1) * 512]
                                if ht_ % 8 in (1, 4, 7):
                                    nc.vector.tensor_copy(osl, op[:])
                                else:
                                    nc.scalar.copy(osl, op[:])
                            r0 = b * 2048 + qo + tc_ * 128
                            # output writes ride the scalar HWDGE queue so
                            # they never block the load queue
                            nc.scalar.dma_start(out_d[r0:r0 + 128, :], os_[:])

                    pending = None
                    for b in range(2):
                        if b == 1:
                            nc.sync.dma_start(vh[:], vh1_dram[:])
                            for hl in range(2):
                                for dc in range(2):
                                    nc.sync.dma_start(
                                        kts[(hl, dc)][:],
                                        qkvT[(4 + 2 * hl + dc, b)][:])
                        for qt in range(4):
                            nkc = 4 * qt + 4  # causal: later k-chunks all-masked
                            qo = qt * 512
                            qmap = {}
                            for hl in range(2):
                                for dc in range(2):
                                    q = qpool.tile([128, 512], F16,
                                                   tag=f"q{hl}{dc}")
                                    nc.sync.dma_start(
                                        q[:],
                                        qkvT[(2 * hl + dc, b)][:, qo:qo + 512])
                                    qmap[(hl, dc)] = q
                            aos = {}
                            for hl in range(2):
                                qs = [qmap[(hl, 0)], qmap[(hl, 1)]]
                                av0 = avpool.tile([128, 512], F32, tag="av0")
                                av1 = avpool.tile([128, 512], F32, tag="av1")
                                den = denpool.tile([128, 512], F32)

                                def emit_av(kc, ex, lo):
                                    nc.tensor.matmul(
                                        av0[:, lo:512],
                                        vh[:, kc * 512 + hl * 256:
                                           kc * 512 + hl * 256 + 128],
                                        ex[:, lo:512],
                                        start=(kc == 0), stop=(kc == nkc - 1))
                                    nc.tensor.matmul(
                                        av1[:, lo:512],
                                        vh[:, kc * 512 + hl * 256 + 128:
                                           kc * 512 + hl * 256 + 256],
                                        ex[:, lo:512],
                                        start=(kc == 0), stop=(kc == nkc - 1))
                                    # denominator, pre-broadcast across
                                    # partitions: ones.T @ ex = colsum x128
                                    nc.tensor.matmul(
                                        den[:, lo:512], onm_sb[:], ex[:, lo:512],
                                        start=(kc == 0), stop=(kc == nkc - 1))

                                pend = []
                                for kc in range(nkc):
                                    # diagonal chunks: columns below lo are
                                    # fully masked and skipped
                                    lo = max(0, (kc - 4 * qt) * 128)
                                    sc = scpool.tile([128, 512], F32)
                                    nc.tensor.matmul(
                                        sc[:, lo:512],
                                        kts[(hl, 0)][:, kc * 128:(kc + 1) * 128],
                                        qs[0][:, lo:512], start=True, stop=False)
                                    nc.tensor.matmul(
                                        sc[:, lo:512],
                                        kts[(hl, 1)][:, kc * 128:(kc + 1) * 128],
                                        qs[1][:, lo:512], start=False, stop=True)
                                    if kc >= 4 * qt:
                                        # triangular mask: 128-wide diag strip
                                        nc.vector.tensor_add(
                                            sc[:, lo:lo + 128],
                                            sc[:, lo:lo + 128],
                                            msk_sb[:, kc - 4 * qt, :])
                                    # two-deep software pipeline: AV for chunk
                                    # kc-2 sits behind the scores of chunk kc
                                    if len(pend) >= 2:
                                        emit_av(*pend.pop(0))
                                    ex = expool.tile([128, 512], F16)
                                    nc.scalar.activation(
                                        ex[:, lo:512], sc[:, lo:512], AF.Exp,
                                        scale=1.0 / 16.0,
                                        bias=kb_sb[:, b * 16 + kc:
                                                   b * 16 + kc + 1])
                                    pend.append((kc, ex, lo))
                                for p in pend:
                                    emit_av(*p)
                                # av-bank evacuation split across ScalarE and
                                # DVE so the banks free fast and the next
                                # block's first AV never waits
                                avc0 = aopool.tile([128, 512], F32, bufs=1,
                                                   tag=f"avs{hl}0", name="avc")
                                nc.scalar.copy(avc0[:], av0[:])
                                avc1 = aopool.tile([128, 512], F32, bufs=1,
                                                   tag=f"avs{hl}1", name="avc")
                                nc.vector.tensor_copy(avc1[:], av1[:])
                                rec = recpool.tile([128, 512], F32, tag="rec",
                                                   bufs=1)
                                nc.vector.reciprocal_approx_fast(rec[:], den[:])
                                for dc, avc in ((0, avc0), (1, avc1)):
                                    ao = aopool.tile([128, 512], F16,
                                                     tag=f"ao{hl}{dc}")
                                    nc.vector.tensor_mul(ao[:], avc[:], rec[:])
                                    aos[(hl, dc)] = ao
                            # emit the PREVIOUS block's out-proj here so its
                            # matmuls sit behind this block's attention in PE
                            # program order and never wait on normalization
                            if pending is not None:
                                emit_outproj(*pending)
                            pending = (b, qt, aos)
                    emit_outproj(*pending)
    nc.compile()
    return nc


def _get_nc():
    if not _NC_CACHE:
        _NC_CACHE.append(_build())
    return _NC_CACHE[0]


def _host_prep(hidden_states, position_ids, attention_mask, w_qkv, w_out):
    hid = np.ascontiguousarray(np.asarray(hidden_states, np.float32)).reshape(TOK, H)
    w_qkv = np.asarray(w_qkv, np.float32)
    w_out = np.asarray(w_out, np.float32)
    pos = np.asarray(position_ids).astype(np.int64)
    am = np.asarray(attention_mask).reshape(B, S).astype(bool)

    # hsT window tiles [w, p, hc*512+t]
    hst = np.ascontiguousarray(
        hid.reshape(8, 512, 32, 128).transpose(0, 3, 2, 1)
    ).reshape(8, 128, 32 * 512).astype(np.float16)

    # rotary tables, matching reference.create_sinusoidal_positions
    inv_freq = 1.0 / 10000 ** (np.arange(0, ROT, 2) / ROT)
    si = np.einsum('i,j->ij', np.arange(MAX_POS), inv_freq).astype('float32')
    emb = np.concatenate([np.sin(si), np.cos(si)], axis=-1)  # [2048, 64]
    sincos = emb[pos]                    # [B, S, 64]
    sin_rep = np.repeat(sincos[..., :ROT // 2], 2, axis=2)   # [B, S, 64]
    cos_rep = np.repeat(sincos[..., ROT // 2:], 2, axis=2)
    rope = np.empty((128, TOK), np.float16)
    rope[0:64] = cos_rep.reshape(TOK, 64).T
    rope[64:128] = sin_rep.reshape(TOK, 64).T

    rt = np.zeros((64, 64), np.float16)
    rt[np.arange(1, 64, 2), np.arange(0, 64, 2)] = -1.0
    rt[np.arange(0, 64, 2), np.arange(1, 64, 2)] = 1.0

    onesm = np.ones((128, 128), np.float16)

    # triangular mask for the 128-wide diagonal strip: key partition p vs
    # query column offset within the strip (identical for every strip)
    p_idx = np.arange(128)[:, None, None]
    q_idx = np.arange(128)[None, None, :]
    masks = np.where(p_idx <= q_idx, 0.0, NEG).astype(np.float16)
    masks = np.broadcast_to(masks, (128, 4, 128)).copy()

    kb = np.where(am.reshape(B, 16, 128), 0.0, NEG).astype(
        np.float32).transpose(2, 0, 1).reshape(128, 32)
    kb = np.ascontiguousarray(kb)

    shared = dict(hst=hst, rope=rope, rt=rt, onesm=onesm, masks=masks, kb=kb)

    in_maps = []
    for c in range(N_CORES):
        qk_cols = []
        v_cols = []
        # fused layout per mp-group is (query, value, key)
        for part, dest in ((0, qk_cols), (2, qk_cols), (1, v_cols)):
            for hl in range(HPC):
                h = HPC * c + hl
                base = (h // 4) * 3072 + part * 1024 + (h % 4) * 256
                dest.append(np.arange(base, base + 256))
        qk_cols = np.concatenate(qk_cols)  # [1024] = q(512) | k(512)
        wslice = w_qkv[:, qk_cols]         # [4096, 1024]
        wqkv_prep = np.ascontiguousarray(
            wslice.reshape(32, 128, 8, 128).transpose(2, 1, 0, 3)
        ).reshape(8, 128, 32 * 128).astype(np.float16)
        v_cols = np.concatenate(v_cols)    # [512]
        wv_prep = np.ascontiguousarray(
            w_qkv[:, v_cols].reshape(32, 128, 512).transpose(1, 0, 2)
        ).reshape(128, 32 * 512).astype(np.float16)
        wout_prep = np.ascontiguousarray(
            w_out[c * DPC:(c + 1) * DPC, :].reshape(4, 128, H).transpose(1, 0, 2)
        ).astype(np.float16)
        in_maps.append(dict(shared, wqkv=wqkv_prep, wv=wv_prep, wout=wout_prep))
    return in_maps


def kernel(hidden_states, position_ids, attention_mask, w_qkv, w_out):
    global LAST_EXEC_NS
    nc = _get_nc()
    in_maps = _host_prep(hidden_states, position_ids, attention_mask,
                         w_qkv, w_out)
    res = run_bass_kernel_spmd(nc, in_maps, core_ids=list(range(N_CORES)))
    LAST_EXEC_NS = res.exec_time_ns
    out = res.results[0]["out"].astype(np.float32)
    for c in range(1, N_CORES):
        out = out + res.results[c]["out"].astype(np.float32)
    return out.reshape(B, S, H)


# revision 22
# speedup vs baseline: 1.2499x; 1.0055x over previous
"""CodeGen-style attention block, tensor-parallel over heads on 8 Trainium2 cores.

Strategy (megatron-style):
  - Each core owns 2 of the 16 heads: computes Q/K/V projections for its
    head-slice of w_qkv, runs causal attention for those heads, then applies
    its row-slice of w_out, producing a partial [tokens, H] output.
  - Host sums the 8 partial outputs (the out-proj contraction over heads).

v5 notes:
  - All matmuls are fp16 (fp32 PSUM accumulate): fp32r moving operands stream
    at ~0.55 ns/col on HW while fp16 hits the full 1 col/cycle rate, halves
    every DMA transfer, and enables fast weight loads.
  - Phase 1 is split: 1a projects K+Q over hidden windows 0..7, 1b projects V
    walking windows 7..0 (reusing window 7's SBUF-resident hidden tile).
    The split frees the K/Q weight space during 1b, which buys room to
    preload w_out and the first q tiles while 1b computes - phase 2 then
    starts with zero DMA waits.
  - V is projected directly into [token, dim] layout (hidden chunks
    stationary, w_v moving) - no PE transposes / DVE copies in phase 2.
    Batch 0's V and K land straight in resident SBUF tiles (no DRAM trip).
  - Attention inner loop is software-pipelined two chunks deep: AV for chunk
    kc trails the scores of chunk kc+2, so ScalarE's exp latency never
    reaches the PE - even on the short diagonal sub-tiles.
  - The softmax denominator rides the PE as one fp16 ones-matmul per chunk.
  - The reciprocal uses the ~5x faster Newton-Raphson DVE variant (4e-6 rel
    err, irrelevant vs fp16 rounding).
  - Diagonal k-chunks only compute the live column subrange [i*128, 512).
  - All stores ride the GpSimd SWDGE queue; the Sync HWDGE queue carries
    only loads, so prefetches never sit behind bulk writes.
"""

import sys
import types
from contextlib import ExitStack

import numpy as np

import concourse.bacc as bacc
import concourse.mybir as mybir
import concourse.tile as tile
from concourse.bass_utils import run_bass_kernel_spmd

# bass_utils imports antenv.axon_hooks when tracing is requested via env;
# provide a no-op stub if the module is absent so a stray BASS_TRACE in the
# environment cannot break execution.
try:
    import antenv.axon_hooks  # noqa: F401
except ImportError:
    _stub = types.ModuleType("antenv.axon_hooks")
    _stub.get_axon_ntff_profile_hook = lambda: None
    _stub.set_axon_ntff_profile_hook = lambda h: None
    sys.modules.setdefault("antenv.axon_hooks", _stub)

F32 = mybir.dt.float32
F16 = mybir.dt.float16
AF = mybir.ActivationFunctionType

B, S, H = 2, 2048, 4096
N_HEAD, HEAD_DIM, ROT = 16, 256, 64
MAX_POS = 2048
TOK = B * S            # 4096
N_CORES = 8
HPC = N_HEAD // N_CORES  # heads per core = 2
DPC = HPC * HEAD_DIM     # dims per core = 512
NEG = -30000.0

LAST_EXEC_NS = None
_NC_CACHE = []


def _build():
    nc = bacc.Bacc("TRN2", target_bir_lowering=False, debug=False,
                   num_devices=N_CORES)

    # [w, p, hc*512+t]: hsT window tiles (512 tokens each), per-partition-contiguous
    hst_d = nc.dram_tensor("hst", [8, 128, 32 * 512], F16, kind="ExternalInput")
    # [oc, p, hc*128+d]: per-core w_qkv column-chunks for q (oc 0-3), k (oc 4-7)
    wqkv_d = nc.dram_tensor("wqkv", [8, 128, 32 * 128], F16, kind="ExternalInput")
    # [p, hc*512+v]: per-core w_v slice, hidden-chunk-major (moving operand)
    wv_d = nc.dram_tensor("wv", [128, 32 * 512], F16, kind="ExternalInput")
    # [p, c, n]: per-core w_out row-slice
    wout_d = nc.dram_tensor("wout", [128, 4, H], F16, kind="ExternalInput")
    rope_d = nc.dram_tensor("rope", [128, TOK], F16, kind="ExternalInput")
    rt_d = nc.dram_tensor("rt", [64, 64], F16, kind="ExternalInput")
    onm_d = nc.dram_tensor("onesm", [128, 128], F16, kind="ExternalInput")
    msk_d = nc.dram_tensor("masks", [128, 4, 128], F16, kind="ExternalInput")
    kb_d = nc.dram_tensor("kb", [128, 32], F32, kind="ExternalInput")
    out_d = nc.dram_tensor("out", [TOK, H], F16, kind="ExternalOutput")

    K_OCS = (4, 5, 6, 7)
    Q_OCS = (0, 1, 2, 3)

    with tile.TileContext(nc) as tc:
        with ExitStack() as st0:
            ec0 = st0.enter_context
            dram_pool = ec0(tc.tile_pool(name="dram", bufs=1, space="DRAM"))
            # DRAM intermediates; batch 0's K and V stay on-chip instead.
            qkvT = {}
            for oc in range(8):
                for b in range(2):
                    if oc >= 4 and b == 0:
                        continue
                    qkvT[(oc, b)] = dram_pool.tile(
                        [128, 2048], F16, tag=f"qkvT{oc}_{b}",
                        name=f"qkvT{oc}_{b}")
            vh1_dram = dram_pool.tile([128, 16 * 512], F16, tag="vh1",
                                      name="vh1")
            # persistent SBUF: resident K (b=0 written by 1a), resident V
            # (b=0 written by 1b), small attention constants, and the shared
            # hidden-window pool reused by 1a and 1b
            kpool = ec0(tc.tile_pool(name="kt", bufs=1))
            kts = {}
            for hl in range(2):
                for dc in range(2):
                    kts[(hl, dc)] = kpool.tile(
                        [128, 2048], F16, tag=f"kt{hl}{dc}", name=f"kt{hl}{dc}")
            vhpool = ec0(tc.tile_pool(name="vh", bufs=1))
            vh = vhpool.tile([128, 16 * 512], F16, tag="vha", name="vha")
            c2 = ec0(tc.tile_pool(name="p2c", bufs=1))
            msk_sb = c2.tile([128, 4, 128], F16)
            kb_sb = c2.tile([128, 32], F32)
            onm_sb = c2.tile([128, 128], F16)
            stH = ec0(ExitStack())
            ecH = stH.enter_context
            # w_v lives in the 1a+1b scope so its load streams in under
            # phase 1a's compute (a 1b-scoped tile would WAR-wait on 1a's
            # freed pools and stall 1b's first matmuls)
            wvpool = ecH(tc.tile_pool(name="wv", bufs=1))
            wv_sb = wvpool.tile([128, 32 * 512], F16, name="wv")
            hpool = ecH(tc.tile_pool(name="ht", bufs=2))

            def ht_load(w, strips=1):
                # strip the transfer so the first H-chunks land (and the
                # first matmuls start) before the whole 4MB tile arrives
                t = hpool.tile([128, 32 * 512], F16, name="ht")
                step = 32 // strips
                for s in range(strips):
                    cs = slice(s * step * 512, (s + 1) * step * 512)
                    nc.sync.dma_start(t[:, cs], hst_d[w][:, cs])
                return t

            # ---------------- Phase 1a: K+Q projection + rotary ----------------
            with ExitStack() as st1:
                ec = st1.enter_context
                cpool = ec(tc.tile_pool(name="p1c", bufs=1))
                wpool = ec(tc.tile_pool(name="w", bufs=1))
                spool = ec(tc.tile_pool(name="stage", bufs=3))
                tpool = ec(tc.tile_pool(name="rott", bufs=1))
                apool = ec(tc.tile_pool(name="acc", bufs=4, space="PSUM"))
                rpool = ec(tc.tile_pool(name="rp", bufs=2, space="PSUM"))
                rope_sb = cpool.tile([128, TOK], F16)
                rt_sb = cpool.tile([64, 64], F16)

                wts = {}

                def load_w(oc, strips=1):
                    wt = wpool.tile([128, 32 * 128], F16, tag=f"w{oc}",
                                    name=f"wt{oc}")
                    step = 32 // strips
                    for s in range(strips):
                        cs = slice(s * step * 128, (s + 1) * step * 128)
                        nc.sync.dma_start(wt[:, cs], wqkv_d[oc][:, cs])
                    wts[oc] = wt

                load_w(K_OCS[0], 4)  # first MMs need it
                ht = ht_load(0, 8)
                for oc in K_OCS[1:] + Q_OCS:
                    load_w(oc)
                nc.sync.dma_start(wv_sb[:], wv_d[:])
                nc.sync.dma_start(rope_sb[:], rope_d[:])
                nc.sync.dma_start(rt_sb[:], rt_d[:])
                # phase-2 constants: needed much later, loaded off the
                # startup critical path
                nc.sync.dma_start(msk_sb[:], msk_d[:])
                nc.sync.dma_start(kb_sb[:], kb_d[:])
                nc.sync.dma_start(onm_sb[:], onm_d[:])

                def project(wt, dest, rot, ws):
                    # dest: [128, 512] fp16 slice (stage tile or resident kt)
                    acc = apool.tile([128, 512], F32)
                    for hc in range(32):
                        nc.tensor.matmul(
                            acc[:], wt[:, hc * 128:(hc + 1) * 128],
                            ht[:, hc * 512:(hc + 1) * 512],
                            start=(hc == 0), stop=(hc == 31))
                    nc.scalar.copy(dest[:], acc[:])
                    if rot:
                        # partial rotary on first 64 dims of this head
                        rp = rpool.tile([64, 512], F32)
                        nc.tensor.matmul(rp[:], rt_sb[:], dest[0:64, :])
                        t1 = tpool.tile([64, 512], F16, tag="t1")
                        nc.vector.tensor_mul(
                            t1[:], acc[0:64, :], rope_sb[0:64, ws])
                        t2 = tpool.tile([64, 512], F16, tag="t2")
                        nc.vector.tensor_mul(
                            t2[:], rp[:], rope_sb[64:128, ws])
                        nc.vector.tensor_add(dest[0:64, :], t1[:], t2[:])

                for w in range(8):
                    if w > 0:
                        ht = next_ht
                    b, wo = w // 4, (w % 4) * 512
                    ws = slice(w * 512, (w + 1) * 512)
                    for j, oc in enumerate(K_OCS):
                        if j == 1 and w < 7:
                            # prefetch next window under this one's compute
                            next_ht = ht_load(w + 1, 2 if w < 2 else 1)
                        rot = oc in (4, 6)
                        if b == 0:
                            hl, dc = (oc - 4) // 2, (oc - 4) % 2
                            project(wts[oc], kts[(hl, dc)][:, wo:wo + 512],
                                    rot, ws)
                        else:
                            stage = spool.tile([128, 512], F16)
                            project(wts[oc], stage, rot, ws)
                            nc.gpsimd.dma_start(
                                qkvT[(oc, b)][:, wo:wo + 512], stage[:])
                    for oc in Q_OCS:
                        stage = spool.tile([128, 512], F16)
                        project(wts[oc], stage, oc in (0, 2), ws)
                        nc.gpsimd.dma_start(
                            qkvT[(oc, b)][:, wo:wo + 512], stage[:])
                ht_w7 = ht

            if True:
                # ---------------- Phase 1b: V projection ----------------
                with ExitStack() as st1b:
                    ec = st1b.enter_context
                    spool2 = ec(tc.tile_pool(name="stage2", bufs=4))
                    apool2 = ec(tc.tile_pool(name="acc2", bufs=4, space="PSUM"))
                    ht = ht_w7  # window 7's hidden tile is still resident
                    for w in range(7, -1, -1):
                        if w != 7:
                            ht = next_ht
                        b = w // 4
                        for tc_ in range(4):
                            if tc_ == 1 and w > 0:
                                next_ht = ht_load(w - 1)
                            acc = apool2.tile([128, 512], F32)
                            for hc in range(32):
                                nc.tensor.matmul(
                                    acc[:],
                                    ht[:, hc * 512 + tc_ * 128:
                                       hc * 512 + (tc_ + 1) * 128],
                                    wv_sb[:, hc * 512:(hc + 1) * 512],
                                    start=(hc == 0), stop=(hc == 31))
                            kc = (w % 4) * 4 + tc_
                            if b == 0:
                                # batch 0's V lands in the resident tile
                                nc.scalar.copy(
                                    vh[:, kc * 512:(kc + 1) * 512], acc[:])
                            else:
                                stage = spool2.tile([128, 512], F16)
                                nc.scalar.copy(stage[:], acc[:])
                                nc.gpsimd.dma_start(
                                    vh1_dram[:, kc * 512:(kc + 1) * 512],
                                    stage[:])

                # wv + hidden-window pools free here; phase 2 reuses the room
                stH.close()

                # ---------------- Phase 2: attention + out-proj ----------------
                with ExitStack() as st2:
                    ec = st2.enter_context
                    c3 = ec(tc.tile_pool(name="p2w", bufs=1))
                    qpool = ec(tc.tile_pool(name="qq", bufs=2))
                    wout_sb = c3.tile([128, 4, H], F16)
                    # out-proj weights stream in column chunks; the first
                    # out-proj block (emitted one qt later) only needs 2MB
                    for ch in range(8):
                        cs = slice(ch * 512, (ch + 1) * 512)
                        nc.sync.dma_start(wout_sb[:, :, cs], wout_d[:, :, cs])
                    expool = ec(tc.tile_pool(name="ex", bufs=6))
                    recpool = ec(tc.tile_pool(name="rec", bufs=1))
                    aopool = ec(tc.tile_pool(name="ao", bufs=2))
                    ospool = ec(tc.tile_pool(name="os", bufs=2))
                    dapool = ec(tc.tile_pool(name="dacc", bufs=2))
                    scpool = ec(tc.tile_pool(name="sc", bufs=2, space="PSUM"))
                    avpool = ec(tc.tile_pool(name="av", bufs=1, space="PSUM"))
                    denpool = ec(tc.tile_pool(name="den", bufs=1, space="PSUM"))
                    oppool = ec(tc.tile_pool(name="op", bufs=3, space="PSUM"))

                    def emit_outproj(b, qt, aos):
                        qo = qt * 512
                        for tc_ in range(4):
                            # full 4096-wide row block staged in SBUF so the
                            # write is ONE descriptor (a per-chunk dma_start
                            # costs ~615ns of engine-sequencer time each)
                            os_ = ospool.tile([128, 4096], F16, tag="os")
                            for ht_ in range(8):
                                op = oppool.tile([128, 512], F32, tag="op")
                                for ci, (hl, dc) in enumerate(
                                        ((0, 0), (0, 1), (1, 0), (1, 1))):
                                    nc.tensor.matmul(
                                        op[:],
                                        aos[(hl, dc)][:, tc_ * 128:(tc_ + 1) * 128],
                                        wout_sb[:, 2 * hl + dc,
                                                ht_ * 512:(ht_ + 1) * 512],
                                        start=(ci == 0), stop=(ci == 3))
                                # split PSUM evacuation 5:3 across the copy
                                # engines (ScalarE takes more; DVE runs the
                                # reciprocal + normalize chain)
                                osl = os_[:, ht_ * 512:(ht_ + 1) * 512]
                                if ht_ % 8 in (1, 4, 7):
                                    nc.vector.tensor_copy(osl, op[:])
                                else:
                                    nc.scalar.copy(osl, op[:])
                            r0 = b * 2048 + qo + tc_ * 128
                            # output writes ride the scalar HWDGE queue so
                            # they never block the load queue
                            nc.scalar.dma_start(out_d[r0:r0 + 128, :], os_[:])

                    pending = None
                    pending_fin = [None]
                    for b in range(2):
                        if b == 1:
                            nc.sync.dma_start(vh[:], vh1_dram[:])
                            for hl in range(2):
                                for dc in range(2):
                                    nc.sync.dma_start(
                                        kts[(hl, dc)][:],
                                        qkvT[(4 + 2 * hl + dc, b)][:])
                        for qt in range(4):
                            nkc = 4 * qt + 4  # causal: later k-chunks all-masked
                            qo = qt * 512
                            qmap = {}
                            for hl in range(2):
                                for dc in range(2):
                                    q = qpool.tile([128, 512], F16,
                                                   tag=f"q{hl}{dc}")
                                    nc.sync.dma_start(
                                        q[:],
                                        qkvT[(2 * hl + dc, b)][:, qo:qo + 512])
                                    qmap[(hl, dc)] = q
                            aos = {}
                            for hl in range(2):
                                qs = [qmap[(hl, 0)], qmap[(hl, 1)]]
                                av0 = avpool.tile([128, 512], F32, tag="av0")
                                av1 = avpool.tile([128, 512], F32, tag="av1")
                                den_acc = dapool.tile([128, 512], F32,
                                                      tag="dna")

                                def emit_av(kc, ex, lo):
                                    nc.tensor.matmul(
                                        av0[:, lo:512],
                                        vh[:, kc * 512 + hl * 256:
                                           kc * 512 + hl * 256 + 128],
                                        ex[:, lo:512],
                                        start=(kc == 0), stop=(kc == nkc - 1))
                                    nc.tensor.matmul(
                                        av1[:, lo:512],
                                        vh[:, kc * 512 + hl * 256 + 128:
                                           kc * 512 + hl * 256 + 256],
                                        ex[:, lo:512],
                                        start=(kc == 0), stop=(kc == nkc - 1))

                                pend = []
                                for kc in range(nkc):
                                    # diagonal chunks: columns below lo are
                                    # fully masked and skipped
                                    lo = max(0, (kc - 4 * qt) * 128)
                                    sc = scpool.tile([128, 512], F32)
                                    nc.tensor.matmul(
                                        sc[:, lo:512],
                                        kts[(hl, 0)][:, kc * 128:(kc + 1) * 128],
                                        qs[0][:, lo:512], start=True, stop=False)
                                    nc.tensor.matmul(
                                        sc[:, lo:512],
                                        kts[(hl, 1)][:, kc * 128:(kc + 1) * 128],
                                        qs[1][:, lo:512], start=False, stop=True)
                                    if kc >= 4 * qt:
                                        # triangular mask: 128-wide diag strip
                                        nc.vector.tensor_add(
                                            sc[:, lo:lo + 128],
                                            sc[:, lo:lo + 128],
                                            msk_sb[:, kc - 4 * qt, :])
                                    # the PREVIOUS block's normalization tail
                                    # emits two chunks in: its DVE chain had a
                                    # whole block of slack, so the reduce
                                    # matmul never stalls the PE
                                    if kc == 2 and pending_fin[0] is not None:
                                        pending_fin[0]()
                                        pending_fin[0] = None
                                    # two-deep software pipeline: AV for chunk
                                    # kc-2 sits behind the scores of chunk kc
                                    if len(pend) >= 2:
                                        emit_av(*pend.pop(0))
                                    ex = expool.tile([128, 512], F16)
                                    nc.scalar.activation(
                                        ex[:, lo:512], sc[:, lo:512], AF.Exp,
                                        scale=1.0 / 16.0,
                                        bias=kb_sb[:, b * 16 + kc:
                                                   b * 16 + kc + 1])
                                    # denominator partials accumulate on DVE,
                                    # off the PE's critical stream
                                    if kc == 0:
                                        nc.vector.tensor_copy(den_acc[:], ex[:])
                                    else:
                                        nc.vector.tensor_add(
                                            den_acc[:, lo:512],
                                            den_acc[:, lo:512], ex[:, lo:512])
                                    pend.append((kc, ex, lo))
                                for p in pend:
                                    emit_av(*p)
                                # av-bank evacuation split across ScalarE and
                                # DVE so the banks free fast and the next
                                # block's first AV never waits
                                avc0 = aopool.tile([128, 512], F32, bufs=1,
                                                   tag=f"avs{hl}0", name="avc")
                                nc.scalar.copy(avc0[:], av0[:])
                                avc1 = aopool.tile([128, 512], F32, bufs=1,
                                                   tag=f"avs{hl}1", name="avc")
                                nc.vector.tensor_copy(avc1[:], av1[:])

                                def finisher(hl=hl, den_acc=den_acc,
                                             avc0=avc0, avc1=avc1, aos=aos):
                                    # partition-reduce the denominator
                                    # partials: fp16 convert once (error
                                    # ~5e-5 after the 128-way sum), then one
                                    # fp16 ones-matmul pre-broadcasts the
                                    # column sums across partitions
                                    dn16 = dapool.tile([128, 512], F16,
                                                       tag="dn16")
                                    nc.vector.tensor_copy(dn16[:], den_acc[:])
                                    den = denpool.tile([128, 512], F32)
                                    nc.tensor.matmul(den[:], onm_sb[:], dn16[:])
                                    rec = recpool.tile([128, 512], F32,
                                                       tag="rec", bufs=1)
                                    nc.vector.reciprocal_approx_fast(
                                        rec[:], den[:])
                                    for dc, avc in ((0, avc0), (1, avc1)):
                                        ao = aopool.tile([128, 512], F16,
                                                         tag=f"ao{hl}{dc}")
                                        nc.vector.tensor_mul(
                                            ao[:], avc[:], rec[:])
                                        aos[(hl, dc)] = ao

                                if pending_fin[0] is not None:
                                    pending_fin[0]()  # safety: never happens
                                pending_fin[0] = finisher
                            # emit the PREVIOUS block's out-proj here so its
                            # matmuls sit behind this block's attention in PE
                            # program order and never wait on normalization
                            if pending is not None:
                                emit_outproj(*pending)
                            pending = (b, qt, aos)
                    pending_fin[0]()
                    emit_outproj(*pending)
    nc.compile()
    return nc


def _get_nc():
    if not _NC_CACHE:
        _NC_CACHE.append(_build())
    return _NC_CACHE[0]


def _host_prep(hidden_states, position_ids, attention_mask, w_qkv, w_out):
    hid = np.ascontiguousarray(np.asarray(hidden_states, np.float32)).reshape(TOK, H)
    w_qkv = np.asarray(w_qkv, np.float32)
    w_out = np.asarray(w_out, np.float32)
    pos = np.asarray(position_ids).astype(np.int64)
    am = np.asarray(attention_mask).reshape(B, S).astype(bool)

    # hsT window tiles [w, p, hc*512+t]
    hst = np.ascontiguousarray(
        hid.reshape(8, 512, 32, 128).transpose(0, 3, 2, 1)
    ).reshape(8, 128, 32 * 512).astype(np.float16)

    # rotary tables, matching reference.create_sinusoidal_positions
    inv_freq = 1.0 / 10000 ** (np.arange(0, ROT, 2) / ROT)
    si = np.einsum('i,j->ij', np.arange(MAX_POS), inv_freq).astype('float32')
    emb = np.concatenate([np.sin(si), np.cos(si)], axis=-1)  # [2048, 64]
    sincos = emb[pos]                    # [B, S, 64]
    sin_rep = np.repeat(sincos[..., :ROT // 2], 2, axis=2)   # [B, S, 64]
    cos_rep = np.repeat(sincos[..., ROT // 2:], 2, axis=2)
    rope = np.empty((128, TOK), np.float16)
    rope[0:64] = cos_rep.reshape(TOK, 64).T
    rope[64:128] = sin_rep.reshape(TOK, 64).T

    rt = np.zeros((64, 64), np.float16)
    rt[np.arange(1, 64, 2), np.arange(0, 64, 2)] = -1.0
    rt[np.arange(0, 64, 2), np.arange(1, 64, 2)] = 1.0

    onesm = np.ones((128, 128), np.float16)

    # triangular mask for the 128-wide diagonal strip: key partition p vs
    # query column offset within the strip (identical for every strip)
    p_idx = np.arange(128)[:, None, None]
    q_idx = np.arange(128)[None, None, :]
    masks = np.where(p_idx <= q_idx, 0.0, NEG).astype(np.float16)
    masks = np.broadcast_to(masks, (128, 4, 128)).copy()

    kb = np.where(am.reshape(B, 16, 128), 0.0, NEG).astype(
        np.float32).transpose(2, 0, 1).reshape(128, 32)
    kb = np.ascontiguousarray(kb)

    shared = dict(hst=hst, rope=rope, rt=rt, onesm=onesm, masks=masks, kb=kb)

    in_maps = []
    for c in range(N_CORES):
        qk_cols = []
        v_cols = []
        # fused layout per mp-group is (query, value, key)
        for part, dest in ((0, qk_cols), (2, qk_cols), (1, v_cols)):
            for hl in range(HPC):
                h = HPC * c + hl
                base = (h // 4) * 3072 + part * 1024 + (h % 4) * 256
                dest.append(np.arange(base, base + 256))
        qk_cols = np.concatenate(qk_cols)  # [1024] = q(512) | k(512)
        wslice = w_qkv[:, qk_cols]         # [4096, 1024]
        wqkv_prep = np.ascontiguousarray(
            wslice.reshape(32, 128, 8, 128).transpose(2, 1, 0, 3)
        ).reshape(8, 128, 32 * 128).astype(np.float16)
        v_cols = np.concatenate(v_cols)    # [512]
        wv_prep = np.ascontiguousarray(
            w_qkv[:, v_cols].reshape(32, 128, 512).transpose(1, 0, 2)
        ).reshape(128, 32 * 512).astype(np.float16)
        wout_prep = np.ascontiguousarray(
            w_out[c * DPC:(c + 1) * DPC, :].reshape(4, 128, H).transpose(1, 0, 2)
        ).astype(np.float16)
        in_maps.append(dict(shared, wqkv=wqkv_prep, wv=wv_prep, wout=wout_prep))
    return in_maps


def kernel(hidden_states, position_ids, attention_mask, w_qkv, w_out):
    global LAST_EXEC_NS
    nc = _get_nc()
    in_maps = _host_prep(hidden_states, position_ids, attention_mask,
                         w_qkv, w_out)
    res = run_bass_kernel_spmd(nc, in_maps, core_ids=list(range(N_CORES)))
    LAST_EXEC_NS = res.exec_time_ns
    out = res.results[0]["out"].astype(np.float32)
    for c in range(1, N_CORES):
        out = out + res.results[c]["out"].astype(np.float32)
    return out.reshape(B, S, H)
